# revision 1
# baseline (speedup 1.0000x reference)
"""Causal self-attention (B=4, T=2048, D=1024, H=16, hd=64) on 8 trn2 cores.

Sharding: 4-way data parallel over batch x 2-way tensor parallel over heads.
Core c handles batch c//2 and heads [8*(c%2), 8*(c%2)+8). Each core computes
its heads' partial contribution to the out-projection (a full [T, D] tensor);
the host sums the two head-group partials per batch and adds b_out.

Per-core dataflow (single NeuronCore, Tile framework):
  1. x [T,D] is PE-transposed tile-by-tile into xT [D,T] (contraction over D
     needs D on partitions; fp32 has no DMA-transpose path).
  2. qT/kT [hd,T] are computed head-PAIR-stacked ([128,T] per pair) with
     w_qkv chunks as stationary; V [T,hd] per head is computed naturally and
     scattered into a [V | 1] layout (65 cols per (head, k-tile)).
  3. Scores are computed TRANSPOSED: S^T[k,q] = K @ Q^T via two row-packed
     K=64 matmuls per (pair, k-tile) -> softmax reduction over k becomes a
     matmul reduction: exp(S^T) (ACT, scale folded in) feeds
     matmul(lhsT=[V|1], rhs=P^T) which accumulates both attn^T [hd,q] and the
     softmax sums (row 64) over all k-tiles in one PSUM group. Causality:
     k-tiles above the diagonal are skipped, diagonal blocks get an
     affine_select zero-mask after exp, partial k-tiles use narrower matmuls.
  4. Normalization: reciprocal of the sums row, broadcast across partitions
     with a K=1 PE outer product, one DVE multiply -> attn^T tiles, which are
     exactly the stationary operand the out-projection needs (no transposes).

All matmuls run in fp32r (1-pass FP22 truncation, bf16-rate); attention
probabilities are bf16 (P in [0,1], accumulated in fp32 PSUM).
"""

import numpy as np

import concourse.bass as bass
import concourse.mybir as mybir
import concourse.tile as tile
from concourse import bacc
from concourse.bass_utils import run_bass_kernel_spmd
from concourse.masks import make_identity

B, T, D = 4, 2048, 1024
H, HD = 16, 64
HPC = 8              # heads per core
PAIRS = HPC // 2
CH = D // 128        # K-chunks for the QKV projection
TG = 512             # T-group / q-group width
SCALE = 1.0 / 8.0    # 1/sqrt(HD)

F32 = mybir.dt.float32
F32R = mybir.dt.float32r
BF16 = mybir.dt.bfloat16
AF = mybir.ActivationFunctionType


def _r(ap):
    return ap.bitcast(F32R)


def build_tile_program(tc, x, wqkv, wout, out, bqkv=None, seq_len=T):
    """Emit the per-core program. seq_len is parametrized for small-scale
    simulation tests; the real kernel uses seq_len=T=2048.

    Emission is software-pipelined: attention for q-group g (latency-bound
    serial chains sT -> exp -> mask -> AV) is interleaved at emission level
    with the transposes + QKV projection of group g+1 (dense PE work), so the
    in-order PE fills attention's dependency stalls with projection matmuls.
    """
    nc = tc.nc
    n_tg = seq_len // TG
    n_tt = seq_len // 128
    with_bias = bqkv is not None

    from contextlib import ExitStack
    with ExitStack() as ctx:
        const = ctx.enter_context(tc.tile_pool(name="const", bufs=1))
        wpool = ctx.enter_context(tc.tile_pool(name="wpool", bufs=1))
        xt_pool = ctx.enter_context(tc.tile_pool(name="xt", bufs=1))
        xn_pool = ctx.enter_context(tc.tile_pool(name="xn", bufs=4))
        qt_pool = ctx.enter_context(tc.tile_pool(name="qt", bufs=2 * PAIRS))
        pt_pool = ctx.enter_context(tc.tile_pool(name="pt", bufs=8))
        at_pool = ctx.enter_context(tc.tile_pool(name="at", bufs=3 * PAIRS))
        tb_pool = ctx.enter_context(tc.tile_pool(name="tb", bufs=1))
        ob_pool = ctx.enter_context(tc.tile_pool(name="ob", bufs=3))
        rc_pool = ctx.enter_context(tc.tile_pool(name="rc", bufs=2))
        mm_ps = ctx.enter_context(tc.tile_pool(name="mmps", bufs=2, space="PSUM"))
        st_ps = ctx.enter_context(tc.tile_pool(name="stps", bufs=4, space="PSUM"))
        av_ps = ctx.enter_context(tc.tile_pool(name="avps", bufs=2, space="PSUM"))
        identity = const.tile([128, 128], F32, tag="ident")
        make_identity(nc, identity[:])
        ones_f = const.tile([128, 64], F32, tag="ones_f")
        nc.vector.memset(ones_f[:], 1.0)
        ones = const.tile([128, 64], F32R, tag="ones")
        nc.vector.tensor_copy(out=ones[:], in_=ones_f[:])
        if with_bias:
            b_sb = const.tile([1, 3 * HPC * HD], F32R, tag="bias")
            nc.sync.dma_start(out=b_sb[:], in_=bqkv.bitcast(F32R)[:])
            ones_row = const.tile([1, TG], F32R, tag="ones_row")
            nc.vector.tensor_copy(
                out=ones_row[:], in_=ones_f[0:1, 0:1].broadcast_to([1, TG]))

        # resident weights; w_out pair-packed: pair p rows [128p, 128p+128).
        # The DMAs are deferred until after group 0's x loads so the first
        # transposes aren't queued behind 8MB of weight traffic.
        w_sb = wpool.tile([128, CH, 3 * HPC * HD], F32R, tag="wqkv")
        wout_sb = [wpool.tile([128, D], F32R, tag=f"woutp{p}", name=f"woutp{p}")
                   for p in range(PAIRS)]

        def load_weights():
            # SWDGE queue (gpsimd): runs in parallel with the sync-engine x
            # loads; q|k column blocks first (consumed first by qt/kT chains)
            for c in range(CH):
                nc.gpsimd.dma_start(
                    out=w_sb[:, c, 0:1024],
                    in_=wqkv.bitcast(F32R)[128 * c:128 * (c + 1), 0:1024])
            for c in range(CH):
                nc.gpsimd.dma_start(
                    out=w_sb[:, c, 1024:1536],
                    in_=wqkv.bitcast(F32R)[128 * c:128 * (c + 1), 1024:1536])
            for p in range(PAIRS):
                nc.gpsimd.dma_start(
                    out=wout_sb[p][:],
                    in_=wout.bitcast(F32R)[128 * p:128 * (p + 1), :])
        # kT per pair, head 2p on partitions [0:64), head 2p+1 on [64:128)
        kts = [wpool.tile([128, seq_len], F32R, tag=f"kt{p}", name=f"kt{p}")
               for p in range(PAIRS)]
        # [V | 1] in bf16: per (head, k-tile) a [128, 65] stationary block
        v1 = wpool.tile([128, HPC, n_tt, HD + 1], BF16, tag="v1")
        nc.vector.memset(v1[:, :, :, HD:HD + 1], 1.0)

        qts_of = {}  # g -> [qt tiles per pair]

        def transpose_units(g):
            """8 units: (t4, chunk-half); each transposes 4 x-blocks."""
            xt = xt_pool.tile([128, CH, TG], F32R, tag="xt", name=f"xt{g}")
            xas = {}
            units = []
            for t4 in range(TG // 128):
                for c0 in (0, 4):
                    def u(t4=t4, c0=c0):
                        if c0 == 0:
                            xa = xn_pool.tile([128, D], F32, tag="xn")
                            row0 = g * TG + t4 * 128
                            nc.sync.dma_start(out=xa[:], in_=x[row0:row0 + 128, :])
                            xas[t4] = xa
                        xa = xas[t4]
                        ps = mm_ps.tile([128, 512], F32, tag="mm")
                        for j in range(4):
                            nc.tensor.transpose(
                                ps[:, 128 * j:128 * (j + 1)],
                                xa[:, 128 * (c0 + j):128 * (c0 + j + 1)],
                                identity[:])
                        nc.vector.tensor_copy(
                            out=xt[:, c0:c0 + 4, 128 * t4:128 * (t4 + 1)],
                            in_=ps[:].rearrange("p (c v) -> p c v", c=4))
                    units.append(u)
            return xt, units

        def qkv_units(g, xt):
            """12 units: 4 qt chains, 4 kT chains, 4 V chains."""
            qts = qts_of.setdefault(g, [])
            units = []

            half = {}

            def qk_chain(p, qk, h):
                # split into two half-chains (finer interleave granularity)
                if h == 0:
                    half[(p, qk)] = mm_ps.tile([128, 512], F32, tag="mm",
                                               name=f"qkps_g{g}_p{p}_{qk}")
                ps = half[(p, qk)]
                col = qk * 512 + p * 128
                for c in range(4 * h, 4 * h + 4):
                    nc.tensor.matmul(
                        ps[:, :TG], w_sb[:, c, col:col + 128], xt[:, c, :],
                        start=(c == 0),
                        stop=(c == CH - 1 and not with_bias))
                if h == 0:
                    return
                if with_bias:
                    nc.tensor.matmul(
                        ps[:, :TG], b_sb[0:1, col:col + 128],
                        ones_row[0:1, :], start=False, stop=True)
                if qk == 0:
                    qt = qt_pool.tile([128, TG], F32R, tag="qt")
                    nc.vector.tensor_copy(out=qt[:], in_=ps[:, :TG])
                    qts.append(qt)
                else:
                    nc.vector.tensor_copy(
                        out=kts[p][:, g * TG:(g + 1) * TG], in_=ps[:, :TG])

            def v_chain(t4):
                tt = g * (TG // 128) + t4
                ps = mm_ps.tile([128, 512], F32, tag="mm")
                for c in range(CH):
                    nc.tensor.matmul(
                        ps[:, :512], xt[:, c, 128 * t4:128 * (t4 + 1)],
                        w_sb[:, c, 1024:1536],
                        start=(c == 0),
                        stop=(c == CH - 1 and not with_bias))
                if with_bias:
                    nc.tensor.matmul(
                        ps[:, :512], ones_row[0:1, 0:128],
                        b_sb[0:1, 1024:1536], start=False, stop=True)
                nc.vector.tensor_copy(
                    out=v1[:, :, tt, 0:HD],
                    in_=ps[:, :512].rearrange("p (h d) -> p h d", h=HPC))

            for p in range(PAIRS):
                for h in range(2):
                    units.append(lambda p=p, h=h: qk_chain(p, 0, h))
            for p in range(PAIRS):
                for h in range(2):
                    units.append(lambda p=p, h=h: qk_chain(p, 1, h))
            for t4 in range(TG // 128):
                units.append(lambda t4=t4: v_chain(t4))
            return units

        def attention_units(g):
            """Per pair: one unit per k-tile (sT+exp+mask, AV carried by one),
            then a normalization unit; finally 8 out-projection units."""
            units = []
            at_pairs = []
            qts = qts_of[g]
            for p in range(PAIRS):
                nkt = 4 * (g + 1)
                state = {}

                def start_pair(p=p, state=state, nkt=nkt):
                    state["ava"] = av_ps.tile([HD + 1, TG], F32, tag="av",
                                              name=f"av_g{g}_p{p}_a")
                    state["avb"] = av_ps.tile([HD + 1, TG], F32, tag="av",
                                              name=f"av_g{g}_p{p}_b")
                    state["carry"] = None

                def kt_unit(kt, pos, p=p, state=state, nkt=nkt,
                            sp=start_pair):
                    if pos == 0:
                        sp()
                    qt = qts[p]
                    ava, avb = state["ava"], state["avb"]
                    rdiag = kt - 4 * g
                    col0 = 128 * rdiag if rdiag > 0 else 0
                    ksl = slice(128 * kt, 128 * (kt + 1))
                    sa = st_ps.tile([128, TG], F32, tag="st")
                    sb = st_ps.tile([128, TG], F32, tag="st")
                    nc.tensor.matmul(
                        sa[:, col0:], kts[p][0:64, ksl], qt[0:64, col0:])
                    nc.tensor.matmul(
                        sb[:, col0:], kts[p][64:128, ksl], qt[64:128, col0:])
                    pta = pt_pool.tile([128, TG], BF16, tag="pt")
                    ptb = pt_pool.tile([128, TG], BF16, tag="pt")
                    nc.scalar.activation(
                        pta[:, col0:], sa[:, col0:], AF.Exp, scale=SCALE)
                    nc.scalar.activation(
                        ptb[:, col0:], sb[:, col0:], AF.Exp, scale=SCALE)
                    if rdiag >= 0:
                        for pt_ in (pta, ptb):
                            # keep P^T[k, q] only where q >= k (within-block)
                            nc.gpsimd.affine_select(
                                out=pt_[:, col0:col0 + 128],
                                in_=pt_[:, col0:col0 + 128],
                                compare_op=mybir.AluOpType.is_ge,
                                fill=0.0, base=0, pattern=[[1, 128]],
                                channel_multiplier=-1)
                    if state["carry"] is not None:
                        state["emit_av"](*state["carry"])
                    state["carry"] = (kt, pos, col0, pta, ptb)

                def emit_av(kt, pos, col0, pta, ptb, p=p, state=state,
                            nkt=nkt):
                    ava, avb = state["ava"], state["avb"]
                    nc.tensor.matmul(
                        ava[0:HD + 1, col0:], v1[:, 2 * p, kt, :],
                        pta[:, col0:], start=(pos == 0), stop=(pos == nkt - 1))
                    nc.tensor.matmul(
                        avb[0:HD + 1, col0:], v1[:, 2 * p + 1, kt, :],
                        ptb[:, col0:], start=(pos == 0), stop=(pos == nkt - 1))

                state["emit_av"] = emit_av

                def norm_unit(p=p, state=state):
                    ava, avb = state["ava"], state["avb"]
                    state["emit_av"](*state["carry"])
                    # 1/sums on DVE, broadcast across partitions via a K=1
                    # outer product on the PE (gpsimd partition_broadcast
                    # mis-handles non-zero base partitions on hardware)
                    rca = rc_pool.tile([HD + 1, TG], F32R, tag="rc")
                    rcb = rc_pool.tile([HD + 1, TG], F32R, tag="rc")
                    with nc.allow_low_precision(
                            reason="fp22 reciprocal feeds fp32r matmul"):
                        nc.vector.reciprocal(rca[HD:HD + 1, :],
                                             ava[HD:HD + 1, :])
                        nc.vector.reciprocal(rcb[HD:HD + 1, :],
                                             avb[HD:HD + 1, :])
                    bca = mm_ps.tile([128, 512], F32, tag="mm")
                    bcb = mm_ps.tile([128, 512], F32, tag="mm")
                    nc.tensor.matmul(bca[0:HD, :TG], ones[64:65, 0:64],
                                     rca[HD:HD + 1, :])
                    nc.tensor.matmul(bcb[0:HD, :TG], ones[64:65, 0:64],
                                     rcb[HD:HD + 1, :])
                    at = at_pool.tile([128, TG], F32R, tag="at")
                    nc.vector.tensor_copy(out=at[0:64, :], in_=ava[0:HD, :])
                    tb = tb_pool.tile([HD, TG], F32R, tag="tb")
                    nc.vector.tensor_copy(out=tb[:], in_=avb[0:HD, :])
                    nc.vector.tensor_mul(at[0:64, :], at[0:64, :],
                                         bca[0:HD, :TG])
                    nc.vector.tensor_mul(tb[:], tb[:], bcb[0:HD, :TG])
                    nc.sync.dma_start(out=at[64:128, :], in_=tb[:])
                    at_pairs.append(at)

                for pos, kt in enumerate(range(nkt)):
                    units.append(lambda kt=kt, pos=pos, f=kt_unit: f(kt, pos))
                units.append(norm_unit)

            def oproj_unit(t4, nh):
                qsl = slice(128 * t4, 128 * (t4 + 1))
                row0 = g * TG + 128 * t4
                ps = mm_ps.tile([128, 512], F32, tag="mm")
                for p in range(PAIRS):
                    nc.tensor.matmul(
                        ps[:, :512], at_pairs[p][:, qsl],
                        wout_sb[p][:, 512 * nh:512 * (nh + 1)],
                        start=(p == 0), stop=(p == PAIRS - 1))
                ob = ob_pool.tile([128, 512], F32, tag="ob")
                nc.vector.tensor_copy(out=ob[:], in_=ps[:, :512])
                nc.sync.dma_start(
                    out=out[row0:row0 + 128, 512 * nh:512 * (nh + 1)],
                    in_=ob[:])

            ounits = []
            for t4 in range(TG // 128):
                for nh in range(2):
                    ounits.append(lambda t4=t4, nh=nh: oproj_unit(t4, nh))
            return units, ounits

        def interleave(a_units, b_units):
            na, nb = len(a_units), len(b_units)
            ia = ib = 0
            while ia < na or ib < nb:
                fa = (na - ia) / na if na else 0.0
                fb = (nb - ib) / nb if nb else 0.0
                if ia < na and (fa >= fb or ib >= nb):
                    a_units[ia]()
                    ia += 1
                else:
                    b_units[ib]()
                    ib += 1

        # prologue: group 0 projection (weight DMAs after group 0's x loads)
        xt0, tunits0 = transpose_units(0)
        for u in tunits0:
            u()
        load_weights()
        for u in qkv_units(0, xt0):
            u()
        # steady state: attention(g) interleaved with transposes(g+1) +
        # projection(g+1); out-projections are deferred up to two groups so
        # the last (largest) attention group still has dense PE fill
        pending_oproj = []  # deferred out-projection unit lists, oldest first
        for g in range(n_tg):
            attn, ounits = attention_units(g)
            fill = []
            if g + 1 < n_tg:
                xt1, tunits = transpose_units(g + 1)
                fill += tunits + qkv_units(g + 1, xt1)
                if g + 2 >= n_tg:  # last-but-one: drain nothing yet
                    pass
                if len(pending_oproj) > 1:
                    fill += pending_oproj.pop(0)
            else:
                while pending_oproj:
                    fill += pending_oproj.pop(0)
            interleave(attn, fill)
            pending_oproj.append(ounits)
        for ou in pending_oproj:
            for u in ou:
                u()


def build_program(with_bias, seq_len=T):
    nc = bacc.Bacc("TRN2", target_bir_lowering=False, debug=False,
                   enable_asserts=False, num_devices=8)
    x = nc.dram_tensor("x", [seq_len, D], F32, kind="ExternalInput").ap()
    wqkv = nc.dram_tensor("wqkv", [D, 3 * HPC * HD], F32,
                          kind="ExternalInput").ap()
    wout = nc.dram_tensor("wout", [HPC * HD, D], F32, kind="ExternalInput").ap()
    out = nc.dram_tensor("out", [seq_len, D], F32, kind="ExternalOutput").ap()
    bqkv = None
    if with_bias:
        bqkv = nc.dram_tensor("bqkv", [1, 3 * HPC * HD], F32,
                              kind="ExternalInput").ap()
    with tile.TileContext(nc) as tc:
        build_tile_program(tc, x, wqkv, wout, out, bqkv, seq_len=seq_len)
    nc.compile()
    return nc


_PROGRAM_CACHE = {}


def _get_program(with_bias):
    if with_bias not in _PROGRAM_CACHE:
        _PROGRAM_CACHE[with_bias] = build_program(with_bias)
    return _PROGRAM_CACHE[with_bias]


def make_in_maps(x, w_qkv, b_qkv, w_out, with_bias):
    """Per-core input dicts: core c -> batch c//2, head group c%2."""
    in_maps = []
    for core in range(8):
        b, gr = divmod(core, 2)
        qc = slice(512 * gr, 512 * (gr + 1))
        kc = slice(D + 512 * gr, D + 512 * (gr + 1))
        vc = slice(2 * D + 512 * gr, 2 * D + 512 * (gr + 1))
        wq = np.ascontiguousarray(
            np.concatenate([w_qkv[:, qc], w_qkv[:, kc], w_qkv[:, vc]], axis=1))
        m = {
            "x": np.ascontiguousarray(x[b]),
            "wqkv": wq,
            "wout": np.ascontiguousarray(w_out[512 * gr:512 * (gr + 1), :]),
        }
        if with_bias:
            m["bqkv"] = np.ascontiguousarray(
                np.concatenate([b_qkv[qc], b_qkv[kc], b_qkv[vc]])
            ).reshape(1, -1)
        in_maps.append(m)
    return in_maps


def kernel(x, w_qkv, b_qkv, w_out, b_out):
    x = np.asarray(x, dtype=np.float32)
    w_qkv = np.asarray(w_qkv, dtype=np.float32)
    b_qkv = np.asarray(b_qkv, dtype=np.float32)
    w_out = np.asarray(w_out, dtype=np.float32)
    b_out = np.asarray(b_out, dtype=np.float32)

    with_bias = bool(np.any(b_qkv))
    nc = _get_program(with_bias)
    in_maps = make_in_maps(x, w_qkv, b_qkv, w_out, with_bias)
    res = run_bass_kernel_spmd(nc, in_maps, core_ids=list(range(8))).results

    out = np.empty((B, T, D), dtype=np.float32)
    for b in range(B):
        out[b] = res[2 * b]["out"] + res[2 * b + 1]["out"] + b_out[None, :]
    return out



# revision 13
# speedup vs baseline: 1.0475x; 1.0475x over previous
"""Causal self-attention (B=4, T=2048, D=1024, H=16, hd=64) on 8 trn2 cores.

Sharding: 4-way data parallel over batch x 2-way tensor parallel over heads.
Core c handles batch c//2 and heads [8*(c%2), 8*(c%2)+8). Each core computes
its heads' partial contribution to the out-projection (a full [T, D] tensor);
the host sums the two head-group partials per batch and adds b_out.

Per-core dataflow (single NeuronCore, Tile framework):
  1. x [T,D] is PE-transposed tile-by-tile into xT [D,T] (bf16 identity so
     the pass-through streams at 1 cycle/row; data bits stay fp32/f32r).
  2. qT/kT [hd,T] are computed head-PAIR-stacked ([128,T] per pair) with
     w_qkv chunks as stationary; V [T,hd] per head is computed naturally and
     scattered into v1 (64 cols per (head, k-tile), bf16).
  3. Scores are computed TRANSPOSED: S^T[k,q] = K @ Q^T via two K=64 matmuls
     per (pair, k-tile) into one [128, 2, 512] two-bank PSUM tile; ONE wide
     exp (ACT, scale folded in) produces P^T for both heads in bf16.
     Causality: k-tiles above the diagonal are skipped, diagonal blocks get
     an affine_select zero-mask after exp, partial k-tiles use narrower
     matmuls.
  4. AV runs NON-transposed: per (head, q-subtile 128) the P^T block is the
     STATIONARY operand and V [128,64] streams, so each matmul costs 64
     moving rows instead of 512. All 8 (head, subtile) accumulators of a
     pair pack into ONE psum bank (zero-on-first-touch after a single
     start=True). Softmax sums ride as 1-column matmuls (P^T stationary,
     ones vector moving) into a shared, gpsimd-zeroed sums bank.
  5. Normalization is a DVE broadcast multiply (reciprocal of sums, then
     attn * recip -> bf16), no PE broadcast needed. The normalized attn
     [q, 512] is PE-transposed (bf16) into the out-projection's stationary
     operand; out = attnT.T @ w_out accumulates over 4 hd-chunks.

All big matmuls run in fp32r (1-pass FP22 truncation, bf16-rate) or bf16;
attention probabilities are bf16 (P in [0,1], accumulated in fp32 PSUM).
"""

import numpy as np

import concourse.bass as bass
import concourse.mybir as mybir
import concourse.tile as tile
from concourse import bacc
from concourse.bass_utils import run_bass_kernel_spmd
from concourse.masks import make_identity

B, T, D = 4, 2048, 1024
H, HD = 16, 64
HPC = 8              # heads per core
PAIRS = HPC // 2
CH = D // 128        # K-chunks for the QKV projection
TG = 512             # T-group / q-group width
SCALE = 1.0 / 8.0    # 1/sqrt(HD)

F32 = mybir.dt.float32
F32R = mybir.dt.float32r
BF16 = mybir.dt.bfloat16
AF = mybir.ActivationFunctionType


def _r(ap):
    return ap.bitcast(F32R)


def build_tile_program(tc, x, wqkv, wout, out, bqkv=None, seq_len=T):
    """Emit the per-core program. seq_len is parametrized for small-scale
    simulation tests; the real kernel uses seq_len=T=2048.

    Emission is software-pipelined: attention for q-group g (latency-bound
    serial chains sT -> exp -> mask -> AV) is interleaved at emission level
    with the transposes + QKV projection of group g+1 (dense PE work), so the
    in-order PE fills attention's dependency stalls with projection matmuls.
    """
    nc = tc.nc
    n_tg = seq_len // TG
    n_tt = seq_len // 128
    with_bias = bqkv is not None

    from contextlib import ExitStack
    with ExitStack() as ctx:
        const = ctx.enter_context(tc.tile_pool(name="const", bufs=1))
        wpool = ctx.enter_context(tc.tile_pool(name="wpool", bufs=1))
        xt_pool = ctx.enter_context(tc.tile_pool(name="xt", bufs=1))
        xn_pool = ctx.enter_context(tc.tile_pool(name="xn", bufs=4))
        qt_pool = ctx.enter_context(tc.tile_pool(name="qt", bufs=2 * PAIRS))
        pt_pool = ctx.enter_context(tc.tile_pool(name="pt", bufs=4))
        an_pool = ctx.enter_context(tc.tile_pool(name="an", bufs=12))
        at_pool = ctx.enter_context(tc.tile_pool(name="at", bufs=4))
        ob_pool = ctx.enter_context(tc.tile_pool(name="ob", bufs=3))
        rc_pool = ctx.enter_context(tc.tile_pool(name="rc", bufs=2))
        mm_ps = ctx.enter_context(tc.tile_pool(name="mmps", bufs=1, space="PSUM"))
        st_ps = ctx.enter_context(tc.tile_pool(name="stps", bufs=2, space="PSUM"))
        av_ps = ctx.enter_context(tc.tile_pool(name="avps", bufs=2, space="PSUM"))
        sums_ps = ctx.enter_context(tc.tile_pool(name="sums", bufs=1, space="PSUM"))
        identity = const.tile([128, 128], F32, tag="ident")
        make_identity(nc, identity[:])
        ident_bf = const.tile([128, 128], BF16, tag="identbf")
        nc.vector.tensor_copy(out=ident_bf[:], in_=identity[:])
        ident_r = const.tile([128, 128], F32R, tag="identr")
        nc.vector.tensor_copy(out=ident_r[:], in_=identity[:])
        ones_f = const.tile([128, 64], F32, tag="ones_f")
        nc.vector.memset(ones_f[:], 1.0)
        ones_bf = const.tile([128, 1], BF16, tag="ones_bf")
        nc.vector.memset(ones_bf[:], 1.0)
        if with_bias:
            b_sb = const.tile([1, 3 * HPC * HD], F32R, tag="bias")
            nc.sync.dma_start(out=b_sb[:], in_=bqkv.bitcast(F32R)[:])
            ones_row = const.tile([1, TG], F32R, tag="ones_row")
            nc.vector.tensor_copy(
                out=ones_row[:], in_=ones_f[0:1, 0:1].broadcast_to([1, TG]))

        # softmax sums: one persistent bank; cols = parity*32 + pair*8 + h*4+s
        sums = sums_ps.tile([128, 64], F32, tag="sums")

        # resident weights; w_out pair-packed: pair p rows [128p, 128p+128).
        # The DMAs are deferred until after group 0's x loads so the first
        # transposes aren't queued behind 8MB of weight traffic.
        w_sb = wpool.tile([128, CH, 3 * HPC * HD], F32R, tag="wqkv")
        wout_sb = [wpool.tile([128, D], BF16, tag=f"woutp{p}", name=f"woutp{p}")
                   for p in range(PAIRS)]

        def load_weights():
            # SWDGE queue (gpsimd): runs in parallel with the sync-engine x
            # loads; q|k column blocks first (consumed first by qt/kT chains)
            for c in range(CH):
                nc.gpsimd.dma_start(
                    out=w_sb[:, c, 0:1024],
                    in_=wqkv.bitcast(F32R)[128 * c:128 * (c + 1), 0:1024])
            for c in range(CH):
                nc.gpsimd.dma_start(
                    out=w_sb[:, c, 1024:1536],
                    in_=wqkv.bitcast(F32R)[128 * c:128 * (c + 1), 1024:1536])
            for p in range(PAIRS):
                nc.gpsimd.dma_start(
                    out=wout_sb[p][:],
                    in_=wout[128 * p:128 * (p + 1), :])
        # kT per pair, head 2p on partitions [0:64), head 2p+1 on [64:128)
        kts = [wpool.tile([128, seq_len], F32R, tag=f"kt{p}", name=f"kt{p}")
               for p in range(PAIRS)]
        # V in bf16: per (head, k-tile) a [128, 64] stationary block
        v1 = wpool.tile([128, HPC, n_tt, HD], BF16, tag="v1")

        qts_of = {}  # g -> [qt tiles per pair]

        def transpose_units(g):
            """8 units: (t4, chunk-half); each transposes 4 x-blocks."""
            xt = xt_pool.tile([128, CH, TG], F32R, tag="xt", name=f"xt{g}")
            xas = {}
            units = []
            for t4 in range(TG // 128):
                for c0 in (0, 4):
                    def u(t4=t4, c0=c0):
                        if c0 == 0:
                            xa = xn_pool.tile([128, D], F32R, tag="xn")
                            row0 = g * TG + t4 * 128
                            nc.sync.dma_start(
                                out=xa[:], in_=_r(x)[row0:row0 + 128, :])
                            xas[t4] = xa
                        xa = xas[t4]
                        ps = mm_ps.tile([128, 512], F32R, tag="mm")
                        for j in range(4):
                            nc.tensor.transpose(
                                ps[:, 128 * j:128 * (j + 1)],
                                xa[:, 128 * (c0 + j):128 * (c0 + j + 1)],
                                ident_r[:])
                        nc.vector.tensor_copy(
                            out=xt[:, c0:c0 + 4, 128 * t4:128 * (t4 + 1)],
                            in_=ps[:].rearrange("p (c v) -> p c v", c=4))
                    units.append(u)
            return xt, units

        def qkv_units(g, xt):
            """12 units: 4 qt chains, 4 kT chains, 4 V chains."""
            qts = qts_of.setdefault(g, [])
            units = []

            half = {}

            def qk_chain(p, qk, h):
                # split into two half-chains (finer interleave granularity)
                if h == 0:
                    half[(p, qk)] = mm_ps.tile([128, 512], F32, tag="mm",
                                               name=f"qkps_g{g}_p{p}_{qk}")
                ps = half[(p, qk)]
                col = qk * 512 + p * 128
                for c in range(4 * h, 4 * h + 4):
                    nc.tensor.matmul(
                        ps[:, :TG], w_sb[:, c, col:col + 128], xt[:, c, :],
                        start=(c == 0),
                        stop=(c == CH - 1 and not with_bias))
                if h == 0:
                    return
                if with_bias:
                    nc.tensor.matmul(
                        ps[:, :TG], b_sb[0:1, col:col + 128],
                        ones_row[0:1, :], start=False, stop=True)
                if qk == 0:
                    qt = qt_pool.tile([128, TG], F32R, tag="qt")
                    nc.vector.tensor_copy(out=qt[:], in_=ps[:, :TG])
                    qts.append(qt)
                else:
                    nc.vector.tensor_copy(
                        out=kts[p][:, g * TG:(g + 1) * TG], in_=ps[:, :TG])

            def v_chain(t4):
                tt = g * (TG // 128) + t4
                ps = mm_ps.tile([128, 512], F32, tag="mm")
                for c in range(CH):
                    nc.tensor.matmul(
                        ps[:, :512], xt[:, c, 128 * t4:128 * (t4 + 1)],
                        w_sb[:, c, 1024:1536],
                        start=(c == 0),
                        stop=(c == CH - 1 and not with_bias))
                if with_bias:
                    nc.tensor.matmul(
                        ps[:, :512], ones_row[0:1, 0:128],
                        b_sb[0:1, 1024:1536], start=False, stop=True)
                nc.vector.tensor_copy(
                    out=v1[:, :, tt, :],
                    in_=ps[:, :512].rearrange("p (h d) -> p h d", h=HPC))

            for p in range(PAIRS):
                for h in range(2):
                    units.append(lambda p=p, h=h: qk_chain(p, 0, h))
            for p in range(PAIRS):
                for h in range(2):
                    units.append(lambda p=p, h=h: qk_chain(p, 1, h))
            for t4 in range(TG // 128):
                units.append(lambda t4=t4: v_chain(t4))
            return units

        def attention_units(g):
            """Per pair: one unit per k-tile (sT+exp+mask, AV carried by one),
            then a normalization unit; finally the out-projection units."""
            units = []
            an_tiles = [an_pool.tile([128, 512], BF16, tag="an",
                                     name=f"an_g{g}_s{s}")
                        for s in range(TG // 128)]
            qts = qts_of[g]
            for p in range(PAIRS):
                nkt = 4 * (g + 1)
                soff = (g % 2) * 32 + p * 8
                state = {}

                def start_pair(p=p, state=state, soff=soff):
                    state["av"] = av_ps.tile([128, 512], F32, tag="av",
                                             name=f"av_g{g}_p{p}")
                    state["carry"] = None
                    state["first"] = True
                    # zero this pair's sums columns (accumulated start=False)
                    nc.vector.memset(sums[:, soff:soff + 8], 0.0)

                def kt_unit(kt, pos, p=p, state=state, nkt=nkt,
                            sp=start_pair):
                    if pos == 0:
                        sp()
                    qt = qts[p]
                    rdiag = kt - 4 * g
                    col0 = 128 * rdiag if rdiag > 0 else 0
                    ksl = slice(128 * kt, 128 * (kt + 1))
                    st = st_ps.tile([128, 2, 512], F32, tag="st")
                    nc.tensor.matmul(
                        st[:, 0, col0:], kts[p][0:64, ksl], qt[0:64, col0:])
                    nc.tensor.matmul(
                        st[:, 1, col0:], kts[p][64:128, ksl], qt[64:128, col0:])
                    pt = pt_pool.tile([128, 2, 512], BF16, tag="pt")
                    nc.scalar.activation(
                        pt[:, :, col0:], st[:, :, col0:], AF.Exp, scale=SCALE)
                    if rdiag >= 0:
                        for h in (0, 1):
                            # keep P^T[k, q] only where q >= k (within-block)
                            nc.gpsimd.affine_select(
                                out=pt[:, h, col0:col0 + 128],
                                in_=pt[:, h, col0:col0 + 128],
                                compare_op=mybir.AluOpType.is_ge,
                                fill=0.0, base=0, pattern=[[1, 128]],
                                channel_multiplier=-1)
                    if state["carry"] is not None:
                        state["emit_av"](*state["carry"])
                    state["carry"] = (kt, pos, rdiag, pt)

                def emit_av(kt, pos, rdiag, pt, p=p, state=state, nkt=nkt,
                            soff=soff):
                    av = state["av"]
                    s0 = max(rdiag, 0)
                    last = pos == nkt - 1
                    for s in range(s0, TG // 128):
                        for h in (0, 1):
                            nc.tensor.matmul(
                                av[:, 128 * s + 64 * h:128 * s + 64 * h + 64],
                                pt[:, h, 128 * s:128 * (s + 1)],
                                v1[:, 2 * p + h, kt, :],
                                start=state["first"],
                                stop=(last and s == 3 and h == 1))
                            state["first"] = False
                            nc.tensor.matmul(
                                sums[:, soff + 4 * h + s:soff + 4 * h + s + 1],
                                pt[:, h, 128 * s:128 * (s + 1)],
                                ones_bf[:],
                                start=False, stop=False,
                                skip_group_check=True)

                state["emit_av"] = emit_av

                def norm_unit(p=p, state=state, soff=soff):
                    av = state["av"]
                    state["emit_av"](*state["carry"])
                    rc = rc_pool.tile([128, 2, 4], F32, tag="rc")
                    nc.vector.reciprocal(
                        rc[:], sums[:, soff:soff + 8].rearrange(
                            "p (h s) -> p h s", h=2))
                    for s in range(TG // 128):
                        nc.vector.tensor_mul(
                            an_tiles[s][:, 128 * p:128 * (p + 1)].rearrange(
                                "p (h d) -> p h d", h=2),
                            av[:, 128 * s:128 * (s + 1)].rearrange(
                                "p (h d) -> p h d", h=2),
                            rc[:, :, s:s + 1].broadcast_to([128, 2, 64]))

                for pos, kt in enumerate(range(nkt)):
                    units.append(lambda kt=kt, pos=pos, f=kt_unit: f(kt, pos))
                units.append(norm_unit)

            ats = {}

            def trans_unit(s):
                ps = mm_ps.tile([128, 512], BF16, tag="mm")
                for c in range(PAIRS):
                    nc.tensor.transpose(
                        ps[:, 128 * c:128 * (c + 1)],
                        an_tiles[s][:, 128 * c:128 * (c + 1)],
                        ident_bf[:])
                at = at_pool.tile([128, 512], BF16, tag="at")
                nc.vector.tensor_copy(out=at[:], in_=ps[:])
                ats[s] = at

            def oproj_unit(s, nh):
                row0 = g * TG + 128 * s
                at = ats[s]
                ps = mm_ps.tile([128, 512], F32, tag="mm")
                for c in range(PAIRS):
                    nc.tensor.matmul(
                        ps[:, :512], at[:, 128 * c:128 * (c + 1)],
                        wout_sb[c][:, 512 * nh:512 * (nh + 1)],
                        start=(c == 0), stop=(c == PAIRS - 1))
                ob = ob_pool.tile([128, 512], F32, tag="ob")
                nc.vector.tensor_copy(out=ob[:], in_=ps[:, :512])
                nc.sync.dma_start(
                    out=out[row0:row0 + 128, 512 * nh:512 * (nh + 1)],
                    in_=ob[:])

            ounits = []
            for s in range(TG // 128):
                ounits.append(lambda s=s: trans_unit(s))
                for nh in range(2):
                    ounits.append(lambda s=s, nh=nh: oproj_unit(s, nh))
            return units, ounits

        def interleave(a_units, b_units):
            na, nb = len(a_units), len(b_units)
            ia = ib = 0
            while ia < na or ib < nb:
                fa = (na - ia) / na if na else 0.0
                fb = (nb - ib) / nb if nb else 0.0
                if ia < na and (fa >= fb or ib >= nb):
                    a_units[ia]()
                    ia += 1
                else:
                    b_units[ib]()
                    ib += 1

        # prologue: group 0 projection (weight DMAs after group 0's x loads)
        xt0, tunits0 = transpose_units(0)
        for u in tunits0:
            u()
        load_weights()
        for u in qkv_units(0, xt0):
            u()
        # steady state: attention(g) interleaved with transposes(g+1) +
        # projection(g+1); out-projections are deferred up to two groups so
        # the last (largest) attention group still has dense PE fill
        pending_oproj = []  # deferred out-projection unit lists, oldest first
        for g in range(n_tg):
            attn, ounits = attention_units(g)
            fill = []
            if g + 1 < n_tg:
                xt1, tunits = transpose_units(g + 1)
                fill += tunits + qkv_units(g + 1, xt1)
                if len(pending_oproj) > 1:
                    fill += pending_oproj.pop(0)
            else:
                while pending_oproj:
                    fill += pending_oproj.pop(0)
            interleave(attn, fill)
            pending_oproj.append(ounits)
        for ou in pending_oproj:
            for u in ou:
                u()


def build_program(with_bias, seq_len=T):
    nc = bacc.Bacc("TRN2", target_bir_lowering=False, debug=False,
                   enable_asserts=False, num_devices=8)
    x = nc.dram_tensor("x", [seq_len, D], F32, kind="ExternalInput").ap()
    wqkv = nc.dram_tensor("wqkv", [D, 3 * HPC * HD], F32,
                          kind="ExternalInput").ap()
    wout = nc.dram_tensor("wout", [HPC * HD, D], BF16, kind="ExternalInput").ap()
    out = nc.dram_tensor("out", [seq_len, D], F32, kind="ExternalOutput").ap()
    bqkv = None
    if with_bias:
        bqkv = nc.dram_tensor("bqkv", [1, 3 * HPC * HD], F32,
                              kind="ExternalInput").ap()
    with tile.TileContext(nc) as tc:
        build_tile_program(tc, x, wqkv, wout, out, bqkv, seq_len=seq_len)
    nc.compile()
    return nc


_PROGRAM_CACHE = {}


def _get_program(with_bias):
    if with_bias not in _PROGRAM_CACHE:
        _PROGRAM_CACHE[with_bias] = build_program(with_bias)
    return _PROGRAM_CACHE[with_bias]


def make_in_maps(x, w_qkv, b_qkv, w_out, with_bias):
    """Per-core input dicts: core c -> batch c//2, head group c%2."""
    in_maps = []
    for core in range(8):
        b, gr = divmod(core, 2)
        qc = slice(512 * gr, 512 * (gr + 1))
        kc = slice(D + 512 * gr, D + 512 * (gr + 1))
        vc = slice(2 * D + 512 * gr, 2 * D + 512 * (gr + 1))
        wq = np.ascontiguousarray(
            np.concatenate([w_qkv[:, qc], w_qkv[:, kc], w_qkv[:, vc]], axis=1))
        import ml_dtypes
        m = {
            "x": np.ascontiguousarray(x[b]),
            "wqkv": wq,
            "wout": np.ascontiguousarray(
                w_out[512 * gr:512 * (gr + 1), :]).astype(ml_dtypes.bfloat16),
        }
        if with_bias:
            m["bqkv"] = np.ascontiguousarray(
                np.concatenate([b_qkv[qc], b_qkv[kc], b_qkv[vc]])
            ).reshape(1, -1)
        in_maps.append(m)
    return in_maps


def kernel(x, w_qkv, b_qkv, w_out, b_out):
    x = np.asarray(x, dtype=np.float32)
    w_qkv = np.asarray(w_qkv, dtype=np.float32)
    b_qkv = np.asarray(b_qkv, dtype=np.float32)
    w_out = np.asarray(w_out, dtype=np.float32)
    b_out = np.asarray(b_out, dtype=np.float32)

    with_bias = bool(np.any(b_qkv))
    nc = _get_program(with_bias)
    in_maps = make_in_maps(x, w_qkv, b_qkv, w_out, with_bias)
    res = run_bass_kernel_spmd(nc, in_maps, core_ids=list(range(8))).results

    out = np.empty((B, T, D), dtype=np.float32)
    for b in range(B):
        out[b] = res[2 * b]["out"] + res[2 * b + 1]["out"] + b_out[None, :]
    return out


# revision 28
# speedup vs baseline: 1.2027x; 1.1483x over previous
"""Causal self-attention (B=4, T=2048, D=1024, H=16, hd=64) on 8 trn2 cores.

Sharding: 4-way data parallel over batch x 2-way tensor parallel over heads.
Core c handles batch c//2 and heads [8*(c%2), 8*(c%2)+8). Each core computes
its heads' partial contribution to the out-projection (a full [T, D] tensor);
the host sums the two head-group partials per batch and adds b_out.

Per-core dataflow (single NeuronCore, Tile framework):
  1. x [T,D] is PE-transposed tile-by-tile into xT [D,T] (bf16 identity so
     the pass-through streams at 1 cycle/row; data bits stay fp32/f32r).
  2. qT/kT [hd,T] are computed head-PAIR-stacked ([128,T] per pair) with
     w_qkv chunks as stationary; V [T,hd] per head is computed naturally and
     scattered into v1 (64 cols per (head, k-tile), bf16).
  3. Scores are computed TRANSPOSED: S^T[k,q] = K @ Q^T via two K=64 matmuls
     per (pair, k-tile) into one [128, 2, 512] two-bank PSUM tile; ONE wide
     exp (ACT, scale folded in) produces P^T for both heads in bf16.
     Causality: k-tiles above the diagonal are skipped, diagonal blocks get
     an affine_select zero-mask after exp, partial k-tiles use narrower
     matmuls.
  4. AV runs NON-transposed: per (head, q-subtile 128) the P^T block is the
     STATIONARY operand and V [128,64] streams, so each matmul costs 64
     moving rows instead of 512. All 8 (head, subtile) accumulators of a
     pair pack into ONE psum bank (zero-on-first-touch after a single
     start=True). Softmax sums ride as 1-column matmuls (P^T stationary,
     ones vector moving) into a shared, gpsimd-zeroed sums bank.
  5. Normalization is a DVE broadcast multiply (reciprocal of sums, then
     attn * recip -> bf16), no PE broadcast needed. The normalized attn
     [q, 512] is PE-transposed (bf16) into the out-projection's stationary
     operand; out = attnT.T @ w_out accumulates over 4 hd-chunks.

All big matmuls run in fp32r (1-pass FP22 truncation, bf16-rate) or bf16;
attention probabilities are bf16 (P in [0,1], accumulated in fp32 PSUM).
"""

import numpy as np

import concourse.bass as bass
import concourse.mybir as mybir
import concourse.tile as tile
from concourse import bacc
from concourse.bass_utils import run_bass_kernel_spmd
from concourse.masks import make_identity

B, T, D = 4, 2048, 1024
H, HD = 16, 64
HPC = 8              # heads per core
PAIRS = HPC // 2
CH = D // 128        # K-chunks for the QKV projection
TG = 512             # T-group / q-group width
SCALE = 1.0 / 8.0    # 1/sqrt(HD)

F32 = mybir.dt.float32
F32R = mybir.dt.float32r
BF16 = mybir.dt.bfloat16
AF = mybir.ActivationFunctionType


def _r(ap):
    return ap.bitcast(F32R)


def build_tile_program(tc, x, wqkv, wout, out, bqkv=None, seq_len=T):
    """Emit the per-core program. seq_len is parametrized for small-scale
    simulation tests; the real kernel uses seq_len=T=2048.

    Emission is software-pipelined: attention for q-group g (latency-bound
    serial chains sT -> exp -> mask -> AV) is interleaved at emission level
    with the transposes + QKV projection of group g+1 (dense PE work), so the
    in-order PE fills attention's dependency stalls with projection matmuls.
    """
    nc = tc.nc
    n_tg = seq_len // TG
    n_tt = seq_len // 128
    with_bias = bqkv is not None

    from contextlib import ExitStack
    with ExitStack() as ctx:
        const = ctx.enter_context(tc.tile_pool(name="const", bufs=1))
        wpool = ctx.enter_context(tc.tile_pool(name="wpool", bufs=1))
        xt_pool = ctx.enter_context(tc.tile_pool(name="xt", bufs=1))
        xn_pool = ctx.enter_context(tc.tile_pool(name="xn", bufs=4))
        qt_pool = ctx.enter_context(tc.tile_pool(name="qt", bufs=2 * PAIRS))
        pt_pool = ctx.enter_context(tc.tile_pool(name="pt", bufs=6))
        an_pool = ctx.enter_context(tc.tile_pool(name="an", bufs=12))
        at_pool = ctx.enter_context(tc.tile_pool(name="at", bufs=4))
        ob_pool = ctx.enter_context(tc.tile_pool(name="ob", bufs=3))
        rc_pool = ctx.enter_context(tc.tile_pool(name="rc", bufs=2))
        mm_ps = ctx.enter_context(tc.tile_pool(name="mmps", bufs=1, space="PSUM"))
        st_ps = ctx.enter_context(tc.tile_pool(name="stps", bufs=2, space="PSUM"))
        av_ps = ctx.enter_context(tc.tile_pool(name="avps", bufs=2, space="PSUM"))
        sums_ps = ctx.enter_context(tc.tile_pool(name="sums", bufs=1, space="PSUM"))
        identity = const.tile([128, 128], F32, tag="ident")
        make_identity(nc, identity[:])
        ident_bf = const.tile([128, 128], BF16, tag="identbf")
        nc.vector.tensor_copy(out=ident_bf[:], in_=identity[:])
        ident_r = const.tile([128, 128], F32R, tag="identr")
        nc.vector.tensor_copy(out=ident_r[:], in_=identity[:])
        ones_f = const.tile([128, 64], F32, tag="ones_f")
        nc.vector.memset(ones_f[:], 1.0)
        ones_bf = const.tile([128, 1], BF16, tag="ones_bf")
        nc.vector.memset(ones_bf[:], 1.0)
        if with_bias:
            b_sb = const.tile([1, 3 * HPC * HD], F32R, tag="bias")
            nc.sync.dma_start(out=b_sb[:], in_=bqkv.bitcast(F32R)[:])
            ones_row = const.tile([1, TG], F32R, tag="ones_row")
            nc.vector.tensor_copy(
                out=ones_row[:], in_=ones_f[0:1, 0:1].broadcast_to([1, TG]))

        # softmax sums: one persistent bank; cols = parity*32 + pair*8 + h*4+s
        sums = sums_ps.tile([128, 64], F32, tag="sums")

        # resident weights; w_out pair-packed: pair p rows [128p, 128p+128).
        # The DMAs are deferred until after group 0's x loads so the first
        # transposes aren't queued behind 8MB of weight traffic.
        w_sb = wpool.tile([128, CH, 3 * HPC * HD], BF16, tag="wqkv")
        wout_sb = [wpool.tile([128, D], BF16, tag=f"woutp{p}", name=f"woutp{p}")
                   for p in range(PAIRS)]

        def load_weights():
            # SWDGE queue (gpsimd): runs in parallel with the sync-engine x
            # loads; q|k column blocks first (consumed first by qt/kT chains)
            for c in range(CH):
                nc.gpsimd.dma_start(
                    out=w_sb[:, c, 0:1024],
                    in_=wqkv[128 * c:128 * (c + 1), 0:1024])
            for c in range(CH):
                nc.gpsimd.dma_start(
                    out=w_sb[:, c, 1024:1536],
                    in_=wqkv[128 * c:128 * (c + 1), 1024:1536])
            for p in range(PAIRS):
                nc.gpsimd.dma_start(
                    out=wout_sb[p][:],
                    in_=wout[128 * p:128 * (p + 1), :])
        # kT per pair, head 2p on partitions [0:64), head 2p+1 on [64:128)
        kts = [wpool.tile([128, seq_len], BF16, tag=f"kt{p}", name=f"kt{p}")
               for p in range(PAIRS)]
        # V in bf16: per (head, k-tile) a [128, 64] stationary block
        v1 = wpool.tile([128, HPC, n_tt, HD], BF16, tag="v1")

        qts_of = {}  # g -> [qt tiles per pair]

        # psum scratch for the projection/transpose/oproj chains. In steady
        # state only the mm bank is free; in the prologue and final drain the
        # scores/AV banks are idle, so rotate through them too (the st slot is
        # [128,2,512]; its first bank is used as a [128,512] scratch).
        _ps_state = {"wide": False, "i": 0}

        def set_wide_scratch(wide):
            _ps_state["wide"] = wide

        def scratch_ps(dtype):
            if not _ps_state["wide"]:
                return mm_ps.tile([128, 512], dtype, tag="mm", name="mm")
            i = _ps_state["i"] = (_ps_state["i"] + 1) % 5
            if i == 0:
                return mm_ps.tile([128, 512], dtype, tag="mm", name="mm")
            if i in (1, 2):
                return st_ps.tile([128, 512], dtype, tag="st", name="stx")
            return av_ps.tile([128, 512], dtype, tag="av", name="avx")

        def transpose_units(g):
            """8 units: (t4, chunk-half); each transposes 4 x-blocks."""
            xt = xt_pool.tile([128, CH, TG], BF16, tag="xt", name=f"xt{g}")
            xas = {}
            units = []

            def prefetch():
                for t4 in range(TG // 128):
                    xa = xn_pool.tile([128, D], BF16, tag="xn",
                                      name=f"xa{g}_{t4}")
                    row0 = g * TG + t4 * 128
                    nc.sync.dma_start(out=xa[:], in_=x[row0:row0 + 128, :])
                    xas[t4] = xa

            for t4 in range(TG // 128):
                for c0 in (0, 4):
                    def u(t4=t4, c0=c0):
                        if t4 == 0 and c0 == 0:
                            prefetch()
                        xa = xas[t4]
                        ps = scratch_ps(BF16)
                        for j in range(4):
                            nc.tensor.transpose(
                                ps[:, 128 * j:128 * (j + 1)],
                                xa[:, 128 * (c0 + j):128 * (c0 + j + 1)],
                                ident_bf[:])
                        nc.vector.tensor_copy(
                            out=xt[:, c0:c0 + 4, 128 * t4:128 * (t4 + 1)],
                            in_=ps[:].rearrange("p (c v) -> p c v", c=4))
                    units.append(u)
            return xt, units

        def qkv_units(g, xt):
            """12 units: 4 qt chains, 4 kT chains, 4 V chains."""
            qts = qts_of.setdefault(g, [])
            units = []

            half = {}

            def qk_chain(p, qk, h):
                # split into two half-chains (finer interleave granularity)
                if h == 0:
                    half[(p, qk)] = scratch_ps(F32)
                ps = half[(p, qk)]
                col = qk * 512 + p * 128
                for c in range(4 * h, 4 * h + 4):
                    nc.tensor.matmul(
                        ps[:, :TG], w_sb[:, c, col:col + 128], xt[:, c, :],
                        start=(c == 0),
                        stop=(c == CH - 1 and not with_bias))
                if h == 0:
                    return
                if with_bias:
                    nc.tensor.matmul(
                        ps[:, :TG], b_sb[0:1, col:col + 128],
                        ones_row[0:1, :], start=False, stop=True)
                if qk == 0:
                    qt = qt_pool.tile([128, TG], BF16, tag="qt")
                    nc.vector.tensor_copy(out=qt[:], in_=ps[:, :TG])
                    qts.append(qt)
                else:
                    nc.vector.tensor_copy(
                        out=kts[p][:, g * TG:(g + 1) * TG], in_=ps[:, :TG])

            def v_chain(t4):
                tt = g * (TG // 128) + t4
                ps = scratch_ps(F32)
                for c in range(CH):
                    nc.tensor.matmul(
                        ps[:, :512], xt[:, c, 128 * t4:128 * (t4 + 1)],
                        w_sb[:, c, 1024:1536],
                        start=(c == 0),
                        stop=(c == CH - 1 and not with_bias))
                if with_bias:
                    nc.tensor.matmul(
                        ps[:, :512], ones_row[0:1, 0:128],
                        b_sb[0:1, 1024:1536], start=False, stop=True)
                nc.vector.tensor_copy(
                    out=v1[:, :, tt, :],
                    in_=ps[:, :512].rearrange("p (h d) -> p h d", h=HPC))

            for p in range(PAIRS):
                for h in range(2):
                    units.append(lambda p=p, h=h: qk_chain(p, 0, h))
            for p in range(PAIRS):
                for h in range(2):
                    units.append(lambda p=p, h=h: qk_chain(p, 1, h))
            for t4 in range(TG // 128):
                units.append(lambda t4=t4: v_chain(t4))
            return units

        def attention_units(g):
            """Per pair: one unit per k-tile (sT+exp+mask, AV carried by one),
            then a normalization unit; finally the out-projection units."""
            units = []
            an_tiles = [an_pool.tile([128, 512], BF16, tag="an",
                                     name=f"an_g{g}_s{s}")
                        for s in range(TG // 128)]
            qts = qts_of[g]
            for p in range(PAIRS):
                nkt = 4 * (g + 1)
                soff = (g % 2) * 32 + p * 8
                state = {}

                def start_pair(p=p, state=state, soff=soff):
                    state["av"] = av_ps.tile([128, 512], F32, tag="av",
                                             name=f"av_g{g}_p{p}")
                    state["carry"] = []
                    state["first"] = True
                    # zero this pair's sums columns (accumulated start=False)
                    nc.vector.memset(sums[:, soff:soff + 8], 0.0)

                def kt_unit(kt, pos, p=p, state=state, nkt=nkt,
                            sp=start_pair):
                    if pos == 0:
                        sp()
                    qt = qts[p]
                    rdiag = kt - 4 * g
                    col0 = 128 * rdiag if rdiag > 0 else 0
                    ksl = slice(128 * kt, 128 * (kt + 1))
                    st = st_ps.tile([128, 2, 512], F32, tag="st")
                    nc.tensor.matmul(
                        st[:, 0, col0:], kts[p][0:64, ksl], qt[0:64, col0:])
                    nc.tensor.matmul(
                        st[:, 1, col0:], kts[p][64:128, ksl], qt[64:128, col0:])
                    pt = pt_pool.tile([128, 2, 512], BF16, tag="pt")
                    nc.scalar.activation(
                        pt[:, :, col0:], st[:, :, col0:], AF.Exp, scale=SCALE)
                    if rdiag >= 0:
                        for h in (0, 1):
                            # keep P^T[k, q] only where q >= k (within-block)
                            nc.gpsimd.affine_select(
                                out=pt[:, h, col0:col0 + 128],
                                in_=pt[:, h, col0:col0 + 128],
                                compare_op=mybir.AluOpType.is_ge,
                                fill=0.0, base=0, pattern=[[1, 128]],
                                channel_multiplier=-1)
                    state["carry"].append((kt, pos, rdiag, pt))
                    if len(state["carry"]) > 2:
                        state["emit_av"](*state["carry"].pop(0))

                def emit_av(kt, pos, rdiag, pt, p=p, state=state, nkt=nkt,
                            soff=soff):
                    av = state["av"]
                    s0 = max(rdiag, 0)
                    last = pos == nkt - 1
                    for s in range(s0, TG // 128):
                        for h in (0, 1):
                            nc.tensor.matmul(
                                av[:, 128 * s + 64 * h:128 * s + 64 * h + 64],
                                pt[:, h, 128 * s:128 * (s + 1)],
                                v1[:, 2 * p + h, kt, :],
                                start=state["first"],
                                stop=(last and s == 3 and h == 1))
                            state["first"] = False
                            nc.tensor.matmul(
                                sums[:, soff + 4 * h + s:soff + 4 * h + s + 1],
                                pt[:, h, 128 * s:128 * (s + 1)],
                                ones_bf[:],
                                start=False, stop=False,
                                skip_group_check=True)

                state["emit_av"] = emit_av

                def norm_unit(p=p, state=state, soff=soff):
                    av = state["av"]
                    while state["carry"]:
                        state["emit_av"](*state["carry"].pop(0))
                    rc = rc_pool.tile([128, 2, 4], F32, tag="rc")
                    nc.vector.reciprocal(
                        rc[:], sums[:, soff:soff + 8].rearrange(
                            "p (h s) -> p h s", h=2))
                    for s in range(TG // 128):
                        nc.vector.tensor_mul(
                            an_tiles[s][:, 128 * p:128 * (p + 1)].rearrange(
                                "p (h d) -> p h d", h=2),
                            av[:, 128 * s:128 * (s + 1)].rearrange(
                                "p (h d) -> p h d", h=2),
                            rc[:, :, s:s + 1].broadcast_to([128, 2, 64]))

                for pos, kt in enumerate(range(nkt)):
                    units.append(lambda kt=kt, pos=pos, f=kt_unit: f(kt, pos))
                units.append(norm_unit)

            ats = {}

            def trans_unit(s):
                ps = scratch_ps(BF16)
                for c in range(PAIRS):
                    nc.tensor.transpose(
                        ps[:, 128 * c:128 * (c + 1)],
                        an_tiles[s][:, 128 * c:128 * (c + 1)],
                        ident_bf[:])
                at = at_pool.tile([128, 512], BF16, tag="at")
                nc.vector.tensor_copy(out=at[:], in_=ps[:])
                ats[s] = at

            def oproj_unit(s, nh):
                row0 = g * TG + 128 * s
                at = ats[s]
                ps = scratch_ps(F32)
                for c in range(PAIRS):
                    nc.tensor.matmul(
                        ps[:, :512], at[:, 128 * c:128 * (c + 1)],
                        wout_sb[c][:, 512 * nh:512 * (nh + 1)],
                        start=(c == 0), stop=(c == PAIRS - 1))
                ob = ob_pool.tile([128, 512], F32, tag="ob")
                nc.vector.tensor_copy(out=ob[:], in_=ps[:, :512])
                nc.sync.dma_start(
                    out=out[row0:row0 + 128, 512 * nh:512 * (nh + 1)],
                    in_=ob[:])

            ounits = []
            for s in range(TG // 128):
                ounits.append(lambda s=s: trans_unit(s))
                for nh in range(2):
                    ounits.append(lambda s=s, nh=nh: oproj_unit(s, nh))
            return units, ounits

        def interleave(a_units, b_units):
            na, nb = len(a_units), len(b_units)
            ia = ib = 0
            while ia < na or ib < nb:
                fa = (na - ia) / na if na else 0.0
                fb = (nb - ib) / nb if nb else 0.0
                if ia < na and (fa >= fb or ib >= nb):
                    a_units[ia]()
                    ia += 1
                else:
                    b_units[ib]()
                    ib += 1

        # prologue: group 0 projection (weight DMAs after group 0's x loads).
        # The scores/AV banks are idle here, so scratch rotates through them.
        set_wide_scratch(True)
        xt0, tunits0 = transpose_units(0)
        for u in tunits0:
            u()
        load_weights()
        for u in qkv_units(0, xt0):
            u()
        set_wide_scratch(False)
        # steady state: attention(g) interleaved with transposes(g+1) +
        # projection(g+1); out-projections are deferred up to two groups so
        # the last (largest) attention group still has dense PE fill
        pending_oproj = []  # deferred out-projection unit lists, oldest first
        for g in range(n_tg):
            attn, ounits = attention_units(g)
            fill = []
            if g + 1 < n_tg:
                xt1, tunits = transpose_units(g + 1)
                fill += tunits + qkv_units(g + 1, xt1)
                if len(pending_oproj) > 1:
                    fill += pending_oproj.pop(0)
            else:
                while pending_oproj:
                    fill += pending_oproj.pop(0)
            interleave(attn, fill)
            pending_oproj.append(ounits)
        # final drain: attention is done, scores/AV banks are idle again
        set_wide_scratch(True)
        for ou in pending_oproj:
            for u in ou:
                u()


def build_program(with_bias, seq_len=T):
    nc = bacc.Bacc("TRN2", target_bir_lowering=False, debug=False,
                   enable_asserts=False, num_devices=8)
    x = nc.dram_tensor("x", [seq_len, D], BF16, kind="ExternalInput").ap()
    wqkv = nc.dram_tensor("wqkv", [D, 3 * HPC * HD], BF16,
                          kind="ExternalInput").ap()
    wout = nc.dram_tensor("wout", [HPC * HD, D], BF16, kind="ExternalInput").ap()
    out = nc.dram_tensor("out", [seq_len, D], F32, kind="ExternalOutput").ap()
    bqkv = None
    if with_bias:
        bqkv = nc.dram_tensor("bqkv", [1, 3 * HPC * HD], F32,
                              kind="ExternalInput").ap()
    with tile.TileContext(nc) as tc:
        build_tile_program(tc, x, wqkv, wout, out, bqkv, seq_len=seq_len)
    nc.compile()
    return nc


_PROGRAM_CACHE = {}


def _get_program(with_bias):
    if with_bias not in _PROGRAM_CACHE:
        _PROGRAM_CACHE[with_bias] = build_program(with_bias)
    return _PROGRAM_CACHE[with_bias]


def make_in_maps(x, w_qkv, b_qkv, w_out, with_bias):
    """Per-core input dicts: core c -> batch c//2, head group c%2."""
    in_maps = []
    for core in range(8):
        b, gr = divmod(core, 2)
        qc = slice(512 * gr, 512 * (gr + 1))
        kc = slice(D + 512 * gr, D + 512 * (gr + 1))
        vc = slice(2 * D + 512 * gr, 2 * D + 512 * (gr + 1))
        wq = np.ascontiguousarray(
            np.concatenate([w_qkv[:, qc], w_qkv[:, kc], w_qkv[:, vc]], axis=1))
        import ml_dtypes
        m = {
            "x": np.ascontiguousarray(x[b]).astype(ml_dtypes.bfloat16),
            "wqkv": wq.astype(ml_dtypes.bfloat16),
            "wout": np.ascontiguousarray(
                w_out[512 * gr:512 * (gr + 1), :]).astype(ml_dtypes.bfloat16),
        }
        if with_bias:
            m["bqkv"] = np.ascontiguousarray(
                np.concatenate([b_qkv[qc], b_qkv[kc], b_qkv[vc]])
            ).reshape(1, -1)
        in_maps.append(m)
    return in_maps


def kernel(x, w_qkv, b_qkv, w_out, b_out):
    x = np.asarray(x, dtype=np.float32)
    w_qkv = np.asarray(w_qkv, dtype=np.float32)
    b_qkv = np.asarray(b_qkv, dtype=np.float32)
    w_out = np.asarray(w_out, dtype=np.float32)
    b_out = np.asarray(b_out, dtype=np.float32)

    with_bias = bool(np.any(b_qkv))
    nc = _get_program(with_bias)
    in_maps = make_in_maps(x, w_qkv, b_qkv, w_out, with_bias)
    res = run_bass_kernel_spmd(nc, in_maps, core_ids=list(range(8))).results

    out = np.empty((B, T, D), dtype=np.float32)
    for b in range(B):
        out[b] = res[2 * b]["out"] + res[2 * b + 1]["out"] + b_out[None, :]
    return out


# revision 41
# speedup vs baseline: 1.2951x; 1.0768x over previous
"""Causal self-attention (B=4, T=2048, D=1024, H=16, hd=64) on 8 trn2 cores.

Sharding: 4-way data parallel over batch x 2-way tensor parallel over heads.
Core c handles batch c//2 and heads [8*(c%2), 8*(c%2)+8). Each core computes
its heads' partial contribution to the out-projection (a full [T, D] tensor);
the host sums the two head-group partials per batch and adds b_out.

Per-core dataflow (single NeuronCore, Tile framework):
  1. x [T,D] is PE-transposed tile-by-tile into xT [D,T] (bf16 identity so
     the pass-through streams at 1 cycle/row; data bits stay fp32/f32r).
  2. qT/kT [hd,T] are computed head-PAIR-stacked ([128,T] per pair) with
     w_qkv chunks as stationary; V [T,hd] per head is computed naturally and
     scattered into v1 (64 cols per (head, k-tile), bf16).
  3. Scores are computed TRANSPOSED: S^T[k,q] = K @ Q^T via two K=64 matmuls
     per (pair, k-tile) into one [128, 2, 512] two-bank PSUM tile; ONE wide
     exp (ACT, scale folded in) produces P^T for both heads in bf16.
     Causality: k-tiles above the diagonal are skipped, diagonal blocks get
     an affine_select zero-mask after exp, partial k-tiles use narrower
     matmuls.
  4. AV runs NON-transposed: per (head, q-subtile 128) the P^T block is the
     STATIONARY operand and V [128,64] streams, so each matmul costs 64
     moving rows instead of 512. All 8 (head, subtile) accumulators of a
     pair pack into ONE psum bank (zero-on-first-touch after a single
     start=True). Softmax sums ride as 1-column matmuls (P^T stationary,
     ones vector moving) into a shared, gpsimd-zeroed sums bank.
  5. Normalization is a DVE broadcast multiply (reciprocal of sums, then
     attn * recip -> bf16), no PE broadcast needed. The normalized attn
     [q, 512] is PE-transposed (bf16) into the out-projection's stationary
     operand; out = attnT.T @ w_out accumulates over 4 hd-chunks.

All big matmuls run in fp32r (1-pass FP22 truncation, bf16-rate) or bf16;
attention probabilities are bf16 (P in [0,1], accumulated in fp32 PSUM).
"""

import numpy as np

import concourse.bass as bass
import concourse.mybir as mybir
import concourse.tile as tile
from concourse import bacc
from concourse.bass_utils import run_bass_kernel_spmd
from concourse.masks import make_identity

B, T, D = 4, 2048, 1024
H, HD = 16, 64
HPC = 8              # heads per core
PAIRS = HPC // 2
CH = D // 128        # K-chunks for the QKV projection
TG = 512             # T-group / q-group width
SCALE = 1.0 / 8.0    # 1/sqrt(HD)

F32 = mybir.dt.float32
F32R = mybir.dt.float32r
BF16 = mybir.dt.bfloat16
AF = mybir.ActivationFunctionType


def _r(ap):
    return ap.bitcast(F32R)


def build_tile_program(tc, x, wqkv, wout, out, bqkv=None, seq_len=T):
    """Emit the per-core program. seq_len is parametrized for small-scale
    simulation tests; the real kernel uses seq_len=T=2048.

    Emission is software-pipelined: attention for q-group g (latency-bound
    serial chains sT -> exp -> mask -> AV) is interleaved at emission level
    with the transposes + QKV projection of group g+1 (dense PE work), so the
    in-order PE fills attention's dependency stalls with projection matmuls.
    """
    nc = tc.nc
    n_tg = seq_len // TG
    n_tt = seq_len // 128
    with_bias = bqkv is not None

    from contextlib import ExitStack
    with ExitStack() as ctx:
        const = ctx.enter_context(tc.tile_pool(name="const", bufs=1))
        wpool = ctx.enter_context(tc.tile_pool(name="wpool", bufs=1))
        xt_pool = ctx.enter_context(tc.tile_pool(name="xt", bufs=2))
        xn_pool = ctx.enter_context(tc.tile_pool(name="xn", bufs=4))
        qt_pool = ctx.enter_context(tc.tile_pool(name="qt", bufs=2 * PAIRS))
        pt_pool = ctx.enter_context(tc.tile_pool(name="pt", bufs=8))
        an_pool = ctx.enter_context(tc.tile_pool(name="an", bufs=16))
        at_pool = ctx.enter_context(tc.tile_pool(name="at", bufs=4))
        ob_pool = ctx.enter_context(tc.tile_pool(name="ob", bufs=3))
        rc_pool = ctx.enter_context(tc.tile_pool(name="rc", bufs=2))
        mm_ps = ctx.enter_context(tc.tile_pool(name="mmps", bufs=1, space="PSUM"))
        st_ps = ctx.enter_context(tc.tile_pool(name="stps", bufs=2, space="PSUM"))
        av_ps = ctx.enter_context(tc.tile_pool(name="avps", bufs=2, space="PSUM"))
        sums_ps = ctx.enter_context(tc.tile_pool(name="sums", bufs=1, space="PSUM"))
        identity = const.tile([128, 128], F32, tag="ident")
        make_identity(nc, identity[:])
        ident_bf = const.tile([128, 128], BF16, tag="identbf")
        nc.vector.tensor_copy(out=ident_bf[:], in_=identity[:])
        ident_r = const.tile([128, 128], F32R, tag="identr")
        nc.vector.tensor_copy(out=ident_r[:], in_=identity[:])
        ones_f = const.tile([128, 64], F32, tag="ones_f")
        nc.vector.memset(ones_f[:], 1.0)
        ones_bf = const.tile([128, 1], BF16, tag="ones_bf")
        nc.vector.memset(ones_bf[:], 1.0)
        if with_bias:
            b_sb = const.tile([1, 3 * HPC * HD], F32R, tag="bias")
            nc.sync.dma_start(out=b_sb[:], in_=bqkv.bitcast(F32R)[:])
            ones_row = const.tile([1, TG], F32R, tag="ones_row")
            nc.vector.tensor_copy(
                out=ones_row[:], in_=ones_f[0:1, 0:1].broadcast_to([1, TG]))

        # softmax sums: one persistent bank; cols = parity*32 + pair*8 + h*4+s
        sums = sums_ps.tile([128, 64], F32, tag="sums")

        # resident weights; w_out pair-packed: pair p rows [128p, 128p+128).
        # The DMAs are deferred until after group 0's x loads so the first
        # transposes aren't queued behind 8MB of weight traffic.
        w_sb = wpool.tile([128, CH, 3 * HPC * HD], BF16, tag="wqkv")
        wout_sb = [wpool.tile([128, D], BF16, tag=f"woutp{p}", name=f"woutp{p}")
                   for p in range(PAIRS)]

        def load_weights():
            # SWDGE queue (gpsimd): runs in parallel with the sync-engine x
            # loads; q|k column blocks first (consumed first by qt/kT chains)
            for c in range(CH):
                nc.gpsimd.dma_start(
                    out=w_sb[:, c, 0:1024],
                    in_=wqkv[128 * c:128 * (c + 1), 0:1024])
            for c in range(CH):
                nc.gpsimd.dma_start(
                    out=w_sb[:, c, 1024:1536],
                    in_=wqkv[128 * c:128 * (c + 1), 1024:1536])
            for p in range(PAIRS):
                nc.gpsimd.dma_start(
                    out=wout_sb[p][:],
                    in_=wout[128 * p:128 * (p + 1), :])
        # kT per pair, head 2p on partitions [0:64), head 2p+1 on [64:128)
        kts = [wpool.tile([128, seq_len], BF16, tag=f"kt{p}", name=f"kt{p}")
               for p in range(PAIRS)]
        # V in bf16: per (head, k-tile) a [128, 64] stationary block
        v1 = wpool.tile([128, HPC, n_tt, HD], BF16, tag="v1")

        qts_of = {}  # g -> [qt tiles per pair]

        # psum scratch for the projection/transpose/oproj chains. In steady
        # state only the mm bank is free; in the prologue and final drain the
        # scores/AV banks are idle, so rotate through them too (the st slot is
        # [128,2,512]; its first bank is used as a [128,512] scratch).
        _ps_state = {"wide": False, "i": 0}

        def set_wide_scratch(wide):
            _ps_state["wide"] = wide

        def scratch_ps(dtype):
            if not _ps_state["wide"]:
                return mm_ps.tile([128, 512], dtype, tag="mm", name="mm")
            i = _ps_state["i"] = (_ps_state["i"] + 1) % 5
            if i == 0:
                return mm_ps.tile([128, 512], dtype, tag="mm", name="mm")
            if i in (1, 2):
                return st_ps.tile([128, 512], dtype, tag="st", name="stx")
            return av_ps.tile([128, 512], dtype, tag="av", name="avx")

        def transpose_units(g):
            """x arrives pre-transposed from the host: just DMA the group's
            xT columns, one [128, TG] block per D-chunk."""
            xt = xt_pool.tile([128, CH, TG], BF16, tag="xt", name=f"xt{g}")

            def u():
                for c in range(CH):
                    nc.sync.dma_start(
                        out=xt[:, c, :],
                        in_=x[128 * c:128 * (c + 1), g * TG:(g + 1) * TG])
            return xt, [u]

        def qkv_units(g, xt):
            """12 units: 4 qt chains, 4 kT chains, 4 V chains."""
            qts = qts_of.setdefault(g, [])
            units = []

            half = {}

            def qk_chain(p, qk, h):
                # split into two half-chains (finer interleave granularity)
                if h == 0:
                    half[(p, qk)] = scratch_ps(F32)
                ps = half[(p, qk)]
                col = qk * 512 + p * 128
                for c in range(4 * h, 4 * h + 4):
                    nc.tensor.matmul(
                        ps[:, :TG], w_sb[:, c, col:col + 128], xt[:, c, :],
                        start=(c == 0),
                        stop=(c == CH - 1 and not with_bias))
                if h == 0:
                    return
                if with_bias:
                    nc.tensor.matmul(
                        ps[:, :TG], b_sb[0:1, col:col + 128],
                        ones_row[0:1, :], start=False, stop=True)
                if qk == 0:
                    qt = qt_pool.tile([128, TG], BF16, tag="qt")
                    nc.vector.tensor_copy(out=qt[:], in_=ps[:, :TG])
                    qts.append(qt)
                else:
                    nc.vector.tensor_copy(
                        out=kts[p][:, g * TG:(g + 1) * TG], in_=ps[:, :TG])

            def v_chain(t4):
                tt = g * (TG // 128) + t4
                ps = scratch_ps(F32)
                for c in range(CH):
                    nc.tensor.matmul(
                        ps[:, :512], xt[:, c, 128 * t4:128 * (t4 + 1)],
                        w_sb[:, c, 1024:1536],
                        start=(c == 0),
                        stop=(c == CH - 1 and not with_bias))
                if with_bias:
                    nc.tensor.matmul(
                        ps[:, :512], ones_row[0:1, 0:128],
                        b_sb[0:1, 1024:1536], start=False, stop=True)
                nc.vector.tensor_copy(
                    out=v1[:, :, tt, :],
                    in_=ps[:, :512].rearrange("p (h d) -> p h d", h=HPC))

            kv_units = []
            for p in range(PAIRS):
                for h in range(2):
                    units.append(lambda p=p, h=h: qk_chain(p, 0, h))
            for p in range(PAIRS):
                for h in range(2):
                    kv_units.append(lambda p=p, h=h: qk_chain(p, 1, h))
            for t4 in range(TG // 128):
                kv_units.append(lambda t4=t4: v_chain(t4))
            return units, kv_units

        def attention_units(g):
            """Per pair: one unit per k-tile (sT+exp+mask, AV carried by one),
            then a normalization unit; finally the out-projection units."""
            units = []
            an_tiles = [an_pool.tile([128, 512], BF16, tag="an",
                                     name=f"an_g{g}_s{s}")
                        for s in range(TG // 128)]
            qts = qts_of[g]
            for p in range(PAIRS):
                nkt = 4 * (g + 1)
                soff = (g % 2) * 32 + p * 8
                state = {}

                def start_pair(p=p, state=state, soff=soff):
                    state["av"] = av_ps.tile([128, 512], F32, tag="av",
                                             name=f"av_g{g}_p{p}")
                    state["carry"] = []
                    state["first"] = True
                    # zero this pair's sums columns (accumulated start=False)
                    nc.vector.memset(sums[:, soff:soff + 8], 0.0)

                def kt_unit(kt, pos, p=p, state=state, nkt=nkt,
                            sp=start_pair):
                    if pos == 0:
                        sp()
                    qt = qts[p]
                    rdiag = kt - 4 * g
                    col0 = 128 * rdiag if rdiag > 0 else 0
                    ksl = slice(128 * kt, 128 * (kt + 1))
                    st = st_ps.tile([128, 2, 512], F32, tag="st")
                    nc.tensor.matmul(
                        st[:, 0, col0:], kts[p][0:64, ksl], qt[0:64, col0:])
                    nc.tensor.matmul(
                        st[:, 1, col0:], kts[p][64:128, ksl], qt[64:128, col0:])
                    pt = pt_pool.tile([128, 2, 512], BF16, tag="pt")
                    nc.scalar.activation(
                        pt[:, :, col0:], st[:, :, col0:], AF.Exp, scale=SCALE)
                    if rdiag >= 0:
                        for h in (0, 1):
                            # keep P^T[k, q] only where q >= k (within-block)
                            nc.gpsimd.affine_select(
                                out=pt[:, h, col0:col0 + 128],
                                in_=pt[:, h, col0:col0 + 128],
                                compare_op=mybir.AluOpType.is_ge,
                                fill=0.0, base=0, pattern=[[1, 128]],
                                channel_multiplier=-1)
                    state["carry"].append((kt, pos, rdiag, pt))
                    if len(state["carry"]) > 3:
                        state["emit_av"](*state["carry"].pop(0))

                def emit_av(kt, pos, rdiag, pt, p=p, state=state, nkt=nkt,
                            soff=soff):
                    av = state["av"]
                    s0 = max(rdiag, 0)
                    last = pos == nkt - 1
                    for s in range(s0, TG // 128):
                        for h in (0, 1):
                            nc.tensor.matmul(
                                av[:, 128 * s + 64 * h:128 * s + 64 * h + 64],
                                pt[:, h, 128 * s:128 * (s + 1)],
                                v1[:, 2 * p + h, kt, :],
                                start=state["first"],
                                stop=(last and s == 3 and h == 1))
                            state["first"] = False
                            nc.tensor.matmul(
                                sums[:, soff + 4 * h + s:soff + 4 * h + s + 1],
                                pt[:, h, 128 * s:128 * (s + 1)],
                                ones_bf[:],
                                start=False, stop=False,
                                skip_group_check=True)

                state["emit_av"] = emit_av

                def norm_unit(p=p, state=state, soff=soff):
                    av = state["av"]
                    while state["carry"]:
                        state["emit_av"](*state["carry"].pop(0))
                    rc = rc_pool.tile([128, 2, 4], F32, tag="rc")
                    nc.vector.reciprocal(
                        rc[:], sums[:, soff:soff + 8].rearrange(
                            "p (h s) -> p h s", h=2))
                    for s in range(TG // 128):
                        nc.vector.tensor_mul(
                            an_tiles[s][:, 128 * p:128 * (p + 1)].rearrange(
                                "p (h d) -> p h d", h=2),
                            av[:, 128 * s:128 * (s + 1)].rearrange(
                                "p (h d) -> p h d", h=2),
                            rc[:, :, s:s + 1].broadcast_to([128, 2, 64]))

                for pos, kt in enumerate(range(nkt)):
                    units.append(lambda kt=kt, pos=pos, f=kt_unit: f(kt, pos))
                units.append(norm_unit)

            ats = {}

            def trans_unit(s):
                ps = scratch_ps(BF16)
                for c in range(PAIRS):
                    nc.tensor.transpose(
                        ps[:, 128 * c:128 * (c + 1)],
                        an_tiles[s][:, 128 * c:128 * (c + 1)],
                        ident_bf[:])
                at = at_pool.tile([128, 512], BF16, tag="at")
                nc.vector.tensor_copy(out=at[:], in_=ps[:])
                ats[s] = at

            def oproj_unit(s, nh):
                row0 = g * TG + 128 * s
                at = ats[s]
                ps = scratch_ps(F32)
                for c in range(PAIRS):
                    nc.tensor.matmul(
                        ps[:, :512], at[:, 128 * c:128 * (c + 1)],
                        wout_sb[c][:, 512 * nh:512 * (nh + 1)],
                        start=(c == 0), stop=(c == PAIRS - 1))
                ob = ob_pool.tile([128, 512], F32, tag="ob")
                if _ps_state["wide"] and (s + nh) % 2 == 0:
                    nc.scalar.copy(ob[:], ps[:, :512])
                else:
                    nc.vector.tensor_copy(out=ob[:], in_=ps[:, :512])
                nc.sync.dma_start(
                    out=out[row0:row0 + 128, 512 * nh:512 * (nh + 1)],
                    in_=ob[:])

            ounits = []
            for s in range(TG // 128):
                ounits.append(lambda s=s: trans_unit(s))
            for s in range(TG // 128):
                for nh in range(2):
                    ounits.append(lambda s=s, nh=nh: oproj_unit(s, nh))
            return units, ounits

        def interleave(a_units, b_units):
            na, nb = len(a_units), len(b_units)
            ia = ib = 0
            while ia < na or ib < nb:
                fa = (na - ia) / na if na else 0.0
                fb = (nb - ib) / nb if nb else 0.0
                if ia < na and (fa >= fb or ib >= nb):
                    a_units[ia]()
                    ia += 1
                else:
                    b_units[ib]()
                    ib += 1

        # prologue: group 0 projection (weight DMAs after group 0's x loads).
        # The scores/AV banks are idle here, so scratch rotates through them.
        set_wide_scratch(True)
        xt0, tunits0 = transpose_units(0)
        for u in tunits0:
            u()
        load_weights()
        q0, kv0 = qkv_units(0, xt0)
        for u in q0 + kv0:
            u()
        set_wide_scratch(False)
        # steady state: attention(g) interleaved with transposes(g+1) +
        # projection(g+1); out-projections are deferred up to two groups so
        # the last (largest) attention group still has dense PE fill
        pending_oproj = []  # deferred out-projection unit lists, oldest first
        kv_pending = []     # group g's own kT/V chains, deferred to phase g
        for g in range(n_tg):
            attn, ounits = attention_units(g)
            fill = []
            if g + 1 < n_tg:
                xt1, tunits = transpose_units(g + 1)
                q1, kv1 = qkv_units(g + 1, xt1)
                fill += tunits + q1 + kv1
            else:
                # last group is exp(ACT)-bound and has no next-group
                # projection: feed it ALL deferred out-projections as fill
                while pending_oproj:
                    fill += pending_oproj.pop(0)
            interleave(attn, fill)
            pending_oproj.append(ounits)
        # final drain: attention is done, scores/AV banks are idle again
        set_wide_scratch(True)
        for ou in pending_oproj:
            for u in ou:
                u()


def build_program(with_bias, seq_len=T):
    nc = bacc.Bacc("TRN2", target_bir_lowering=False, debug=False,
                   enable_asserts=False, num_devices=8)
    x = nc.dram_tensor("xt", [D, seq_len], BF16, kind="ExternalInput").ap()
    wqkv = nc.dram_tensor("wqkv", [D, 3 * HPC * HD], BF16,
                          kind="ExternalInput").ap()
    wout = nc.dram_tensor("wout", [HPC * HD, D], BF16, kind="ExternalInput").ap()
    out = nc.dram_tensor("out", [seq_len, D], F32, kind="ExternalOutput").ap()
    bqkv = None
    if with_bias:
        bqkv = nc.dram_tensor("bqkv", [1, 3 * HPC * HD], F32,
                              kind="ExternalInput").ap()
    with tile.TileContext(nc) as tc:
        build_tile_program(tc, x, wqkv, wout, out, bqkv, seq_len=seq_len)
    nc.compile()
    return nc


_PROGRAM_CACHE = {}


def _get_program(with_bias):
    if with_bias not in _PROGRAM_CACHE:
        _PROGRAM_CACHE[with_bias] = build_program(with_bias)
    return _PROGRAM_CACHE[with_bias]


def make_in_maps(x, w_qkv, b_qkv, w_out, with_bias):
    """Per-core input dicts: core c -> batch c//2, head group c%2."""
    in_maps = []
    for core in range(8):
        b, gr = divmod(core, 2)
        qc = slice(512 * gr, 512 * (gr + 1))
        kc = slice(D + 512 * gr, D + 512 * (gr + 1))
        vc = slice(2 * D + 512 * gr, 2 * D + 512 * (gr + 1))
        wq = np.ascontiguousarray(
            np.concatenate([w_qkv[:, qc], w_qkv[:, kc], w_qkv[:, vc]], axis=1))
        import ml_dtypes
        m = {
            "xt": np.ascontiguousarray(x[b].T).astype(ml_dtypes.bfloat16),
            "wqkv": wq.astype(ml_dtypes.bfloat16),
            "wout": np.ascontiguousarray(
                w_out[512 * gr:512 * (gr + 1), :]).astype(ml_dtypes.bfloat16),
        }
        if with_bias:
            m["bqkv"] = np.ascontiguousarray(
                np.concatenate([b_qkv[qc], b_qkv[kc], b_qkv[vc]])
            ).reshape(1, -1)
        in_maps.append(m)
    return in_maps


def kernel(x, w_qkv, b_qkv, w_out, b_out):
    x = np.asarray(x, dtype=np.float32)
    w_qkv = np.asarray(w_qkv, dtype=np.float32)
    b_qkv = np.asarray(b_qkv, dtype=np.float32)
    w_out = np.asarray(w_out, dtype=np.float32)
    b_out = np.asarray(b_out, dtype=np.float32)

    with_bias = bool(np.any(b_qkv))
    nc = _get_program(with_bias)
    in_maps = make_in_maps(x, w_qkv, b_qkv, w_out, with_bias)
    res = run_bass_kernel_spmd(nc, in_maps, core_ids=list(range(8))).results

    out = np.empty((B, T, D), dtype=np.float32)
    for b in range(B):
        out[b] = res[2 * b]["out"] + res[2 * b + 1]["out"] + b_out[None, :]
    return out


# revision 52
# speedup vs baseline: 1.3292x; 1.0263x over previous
"""Causal self-attention (B=4, T=2048, D=1024, H=16, hd=64) on 8 trn2 cores.

Sharding: 4-way data parallel over batch x 2-way tensor parallel over heads.
Core c handles batch c//2 and heads [8*(c%2), 8*(c%2)+8). Each core computes
its heads' partial contribution to the out-projection (a full [T, D] tensor);
the host sums the two head-group partials per batch and adds b_out.

Per-core dataflow (single NeuronCore, Tile framework):
  1. x [T,D] is PE-transposed tile-by-tile into xT [D,T] (bf16 identity so
     the pass-through streams at 1 cycle/row; data bits stay fp32/f32r).
  2. qT/kT [hd,T] are computed head-PAIR-stacked ([128,T] per pair) with
     w_qkv chunks as stationary; V [T,hd] per head is computed naturally and
     scattered into v1 (64 cols per (head, k-tile), bf16).
  3. Scores are computed TRANSPOSED: S^T[k,q] = K @ Q^T via two K=64 matmuls
     per (pair, k-tile) into one [128, 2, 512] two-bank PSUM tile; ONE wide
     exp (ACT, scale folded in) produces P^T for both heads in bf16.
     Causality: k-tiles above the diagonal are skipped, diagonal blocks get
     an affine_select zero-mask after exp, partial k-tiles use narrower
     matmuls.
  4. AV runs NON-transposed: per (head, q-subtile 128) the P^T block is the
     STATIONARY operand and V [128,64] streams, so each matmul costs 64
     moving rows instead of 512. All 8 (head, subtile) accumulators of a
     pair pack into ONE psum bank (zero-on-first-touch after a single
     start=True). Softmax sums ride as 1-column matmuls (P^T stationary,
     ones vector moving) into a shared, gpsimd-zeroed sums bank.
  5. Normalization is a DVE broadcast multiply (reciprocal of sums, then
     attn * recip -> bf16), no PE broadcast needed. The normalized attn
     [q, 512] is PE-transposed (bf16) into the out-projection's stationary
     operand; out = attnT.T @ w_out accumulates over 4 hd-chunks.

All big matmuls run in fp32r (1-pass FP22 truncation, bf16-rate) or bf16;
attention probabilities are bf16 (P in [0,1], accumulated in fp32 PSUM).
"""

import numpy as np

import concourse.bass as bass
import concourse.mybir as mybir
import concourse.tile as tile
from concourse import bacc
from concourse.bass_utils import run_bass_kernel_spmd
from concourse.masks import make_identity

B, T, D = 4, 2048, 1024
H, HD = 16, 64
HPC = 8              # heads per core
PAIRS = HPC // 2
CH = D // 128        # K-chunks for the QKV projection
TG = 512             # T-group / q-group width
SCALE = 1.0 / 8.0    # 1/sqrt(HD)

F32 = mybir.dt.float32
F32R = mybir.dt.float32r
BF16 = mybir.dt.bfloat16
AF = mybir.ActivationFunctionType


def _r(ap):
    return ap.bitcast(F32R)


def build_tile_program(tc, x, wqkv, wout, out, bqkv=None, seq_len=T):
    """Emit the per-core program. seq_len is parametrized for small-scale
    simulation tests; the real kernel uses seq_len=T=2048.

    Emission is software-pipelined: attention for q-group g (latency-bound
    serial chains sT -> exp -> mask -> AV) is interleaved at emission level
    with the transposes + QKV projection of group g+1 (dense PE work), so the
    in-order PE fills attention's dependency stalls with projection matmuls.
    """
    nc = tc.nc
    n_tg = seq_len // TG
    n_tt = seq_len // 128
    with_bias = bqkv is not None

    from contextlib import ExitStack
    with ExitStack() as ctx:
        const = ctx.enter_context(tc.tile_pool(name="const", bufs=1))
        wpool = ctx.enter_context(tc.tile_pool(name="wpool", bufs=1))
        xt_pool = ctx.enter_context(tc.tile_pool(name="xt", bufs=2))
        xn_pool = ctx.enter_context(tc.tile_pool(name="xn", bufs=4))
        qt_pool = ctx.enter_context(tc.tile_pool(name="qt", bufs=2 * PAIRS))
        pt_pool = ctx.enter_context(tc.tile_pool(name="pt", bufs=14))
        an_pool = ctx.enter_context(tc.tile_pool(name="an", bufs=16))
        at_pool = ctx.enter_context(tc.tile_pool(name="at", bufs=4))
        ob_pool = ctx.enter_context(tc.tile_pool(name="ob", bufs=6))
        rc_pool = ctx.enter_context(tc.tile_pool(name="rc", bufs=2))
        mm_ps = ctx.enter_context(tc.tile_pool(name="mmps", bufs=1, space="PSUM"))
        st_ps = ctx.enter_context(tc.tile_pool(name="stps", bufs=2, space="PSUM"))
        av_ps = ctx.enter_context(tc.tile_pool(name="avps", bufs=2, space="PSUM"))
        sums_ps = ctx.enter_context(tc.tile_pool(name="sums", bufs=1, space="PSUM"))
        identity = const.tile([128, 128], F32, tag="ident")
        make_identity(nc, identity[:])
        ident_bf = const.tile([128, 128], BF16, tag="identbf")
        nc.vector.tensor_copy(out=ident_bf[:], in_=identity[:])
        ident_r = const.tile([128, 128], F32R, tag="identr")
        nc.vector.tensor_copy(out=ident_r[:], in_=identity[:])
        ones_f = const.tile([128, 64], F32, tag="ones_f")
        nc.vector.memset(ones_f[:], 1.0)
        ones_bf = const.tile([128, 1], BF16, tag="ones_bf")
        nc.vector.memset(ones_bf[:], 1.0)
        if with_bias:
            b_sb = const.tile([1, 3 * HPC * HD], F32R, tag="bias")
            nc.sync.dma_start(out=b_sb[:], in_=bqkv.bitcast(F32R)[:])
            ones_row = const.tile([1, TG], F32R, tag="ones_row")
            nc.vector.tensor_copy(
                out=ones_row[:], in_=ones_f[0:1, 0:1].broadcast_to([1, TG]))

        # softmax sums: one persistent bank; cols = parity*32 + pair*8 + h*4+s
        sums = sums_ps.tile([128, 64], F32, tag="sums")

        # resident weights; w_out pair-packed: pair p rows [128p, 128p+128).
        # The DMAs are deferred until after group 0's x loads so the first
        # transposes aren't queued behind 8MB of weight traffic.
        w_sb = wpool.tile([128, CH, 3 * HPC * HD], BF16, tag="wqkv")
        wout_sb = [wpool.tile([128, D], BF16, tag=f"woutp{p}", name=f"woutp{p}")
                   for p in range(PAIRS)]

        def load_weights():
            # SWDGE queue (gpsimd): runs in parallel with the sync-engine x
            # loads; q|k column blocks first (consumed first by qt/kT chains)
            for c in range(CH):
                nc.gpsimd.dma_start(
                    out=w_sb[:, c, 0:1024],
                    in_=wqkv[128 * c:128 * (c + 1), 0:1024])
            for c in range(CH):
                nc.gpsimd.dma_start(
                    out=w_sb[:, c, 1024:1536],
                    in_=wqkv[128 * c:128 * (c + 1), 1024:1536])
            for p in range(PAIRS):
                nc.gpsimd.dma_start(
                    out=wout_sb[p][:],
                    in_=wout[128 * p:128 * (p + 1), :])
        # kT per pair, head 2p on partitions [0:64), head 2p+1 on [64:128)
        kts = [wpool.tile([128, seq_len], BF16, tag=f"kt{p}", name=f"kt{p}")
               for p in range(PAIRS)]
        # V in bf16: per (head, k-tile) a [128, 64] stationary block
        v1 = wpool.tile([128, HPC, n_tt, HD], BF16, tag="v1")

        qts_of = {}  # g -> [qt tiles per pair]

        # psum scratch for the projection/transpose/oproj chains. In steady
        # state only the mm bank is free; in the prologue and final drain the
        # scores/AV banks are idle, so rotate through them too (the st slot is
        # [128,2,512]; its first bank is used as a [128,512] scratch).
        _ps_state = {"wide": False, "i": 0}

        def set_wide_scratch(wide):
            _ps_state["wide"] = wide

        def scratch_ps(dtype):
            if not _ps_state["wide"]:
                return mm_ps.tile([128, 512], dtype, tag="mm", name="mm")
            i = _ps_state["i"] = (_ps_state["i"] + 1) % 5
            if i == 0:
                return mm_ps.tile([128, 512], dtype, tag="mm", name="mm")
            if i in (1, 2):
                return st_ps.tile([128, 512], dtype, tag="st", name="stx")
            return av_ps.tile([128, 512], dtype, tag="av", name="avx")

        def transpose_units(g):
            """x arrives pre-transposed from the host: just DMA the group's
            xT columns, one [128, TG] block per D-chunk."""
            xt = xt_pool.tile([128, CH, TG], BF16, tag="xt", name=f"xt{g}")

            def u():
                for c in range(CH):
                    nc.sync.dma_start(
                        out=xt[:, c, :],
                        in_=x[128 * c:128 * (c + 1), g * TG:(g + 1) * TG])
            return xt, [u]

        def qkv_units(g, xt):
            """12 units: 4 qt chains, 4 kT chains, 4 V chains."""
            qts = qts_of.setdefault(g, [])
            units = []

            half = {}

            def qk_chain(p, qk, h):
                # split into two half-chains (finer interleave granularity)
                if h == 0:
                    half[(p, qk)] = scratch_ps(F32)
                ps = half[(p, qk)]
                col = qk * 512 + p * 128
                for c in range(4 * h, 4 * h + 4):
                    nc.tensor.matmul(
                        ps[:, :TG], w_sb[:, c, col:col + 128], xt[:, c, :],
                        start=(c == 0),
                        stop=(c == CH - 1 and not with_bias))
                if h == 0:
                    return
                if with_bias:
                    nc.tensor.matmul(
                        ps[:, :TG], b_sb[0:1, col:col + 128],
                        ones_row[0:1, :], start=False, stop=True)
                if qk == 0:
                    qt = qt_pool.tile([128, TG], BF16, tag="qt")
                    nc.vector.tensor_copy(out=qt[:], in_=ps[:, :TG])
                    qts.append(qt)
                else:
                    nc.vector.tensor_copy(
                        out=kts[p][:, g * TG:(g + 1) * TG], in_=ps[:, :TG])

            def v_chain(t4):
                tt = g * (TG // 128) + t4
                ps = scratch_ps(F32)
                for c in range(CH):
                    nc.tensor.matmul(
                        ps[:, :512], xt[:, c, 128 * t4:128 * (t4 + 1)],
                        w_sb[:, c, 1024:1536],
                        start=(c == 0),
                        stop=(c == CH - 1 and not with_bias))
                if with_bias:
                    nc.tensor.matmul(
                        ps[:, :512], ones_row[0:1, 0:128],
                        b_sb[0:1, 1024:1536], start=False, stop=True)
                nc.vector.tensor_copy(
                    out=v1[:, :, tt, :],
                    in_=ps[:, :512].rearrange("p (h d) -> p h d", h=HPC))

            kv_units = []
            for p in range(PAIRS):
                for h in range(2):
                    units.append(lambda p=p, h=h: qk_chain(p, 0, h))
            for p in range(PAIRS):
                for h in range(2):
                    kv_units.append(lambda p=p, h=h: qk_chain(p, 1, h))
            for t4 in range(TG // 128):
                kv_units.append(lambda t4=t4: v_chain(t4))
            return units, kv_units

        def attention_units(g):
            """Per pair: one unit per k-tile (sT+exp+mask, AV carried by one),
            then a normalization unit; finally the out-projection units."""
            units = []
            an_tiles = [an_pool.tile([128, 512], BF16, tag="an",
                                     name=f"an_g{g}_s{s}")
                        for s in range(TG // 128)]
            qts = qts_of[g]
            for p in range(PAIRS):
                nkt = 4 * (g + 1)
                soff = (g % 2) * 32 + p * 8
                state = {}

                def start_pair(p=p, state=state, soff=soff):
                    state["av"] = av_ps.tile([128, 512], F32, tag="av",
                                             name=f"av_g{g}_p{p}")
                    state["carry"] = []
                    state["first"] = True
                    # zero this pair's sums columns (accumulated start=False)
                    nc.vector.memset(sums[:, soff:soff + 8], 0.0)

                def kt_unit(kt, pos, p=p, state=state, nkt=nkt,
                            sp=start_pair):
                    if pos == 0:
                        sp()
                    qt = qts[p]
                    rdiag = kt - 4 * g
                    col0 = 128 * rdiag if rdiag > 0 else 0
                    ksl = slice(128 * kt, 128 * (kt + 1))
                    st = st_ps.tile([128, 2, 512], F32, tag="st")
                    nc.tensor.matmul(
                        st[:, 0, col0:], kts[p][0:64, ksl], qt[0:64, col0:])
                    nc.tensor.matmul(
                        st[:, 1, col0:], kts[p][64:128, ksl], qt[64:128, col0:])
                    pt = pt_pool.tile([128, 2, 512], BF16, tag="pt")
                    nc.scalar.activation(
                        pt[:, :, col0:], st[:, :, col0:], AF.Exp, scale=SCALE)
                    if rdiag >= 0:
                        for h in (0, 1):
                            # keep P^T[k, q] only where q >= k (within-block)
                            nc.gpsimd.affine_select(
                                out=pt[:, h, col0:col0 + 128],
                                in_=pt[:, h, col0:col0 + 128],
                                compare_op=mybir.AluOpType.is_ge,
                                fill=0.0, base=0, pattern=[[1, 128]],
                                channel_multiplier=-1)
                    state["carry"].append((kt, pos, rdiag, pt))
                    if len(state["carry"]) > 8:
                        state["emit_av"](*state["carry"].pop(0))

                def emit_av(kt, pos, rdiag, pt, p=p, state=state, nkt=nkt,
                            soff=soff):
                    av = state["av"]
                    s0 = max(rdiag, 0)
                    last = pos == nkt - 1
                    for s in range(s0, TG // 128):
                        for h in (0, 1):
                            nc.tensor.matmul(
                                av[:, 128 * s + 64 * h:128 * s + 64 * h + 64],
                                pt[:, h, 128 * s:128 * (s + 1)],
                                v1[:, 2 * p + h, kt, :],
                                start=state["first"],
                                stop=(last and s == 3 and h == 1))
                            state["first"] = False
                            nc.tensor.matmul(
                                sums[:, soff + 4 * h + s:soff + 4 * h + s + 1],
                                pt[:, h, 128 * s:128 * (s + 1)],
                                ones_bf[:],
                                start=False, stop=False,
                                skip_group_check=True)

                state["emit_av"] = emit_av

                def norm_unit(p=p, state=state, soff=soff):
                    av = state["av"]
                    while state["carry"]:
                        state["emit_av"](*state["carry"].pop(0))
                    rc = rc_pool.tile([128, 2, 4], F32, tag="rc")
                    nc.vector.reciprocal(
                        rc[:], sums[:, soff:soff + 8].rearrange(
                            "p (h s) -> p h s", h=2))
                    for s in range(TG // 128):
                        nc.vector.tensor_mul(
                            an_tiles[s][:, 128 * p:128 * (p + 1)].rearrange(
                                "p (h d) -> p h d", h=2),
                            av[:, 128 * s:128 * (s + 1)].rearrange(
                                "p (h d) -> p h d", h=2),
                            rc[:, :, s:s + 1].broadcast_to([128, 2, 64]))

                for pos, kt in enumerate(range(nkt)):
                    units.append(lambda kt=kt, pos=pos, f=kt_unit: f(kt, pos))
                units.append(norm_unit)

            ats = {}

            def trans_unit(s):
                ps = scratch_ps(BF16)
                for c in range(PAIRS):
                    nc.tensor.transpose(
                        ps[:, 128 * c:128 * (c + 1)],
                        an_tiles[s][:, 128 * c:128 * (c + 1)],
                        ident_bf[:])
                at = at_pool.tile([128, 512], BF16, tag="at")
                nc.vector.tensor_copy(out=at[:], in_=ps[:])
                ats[s] = at

            def oproj_unit(s, nh):
                row0 = g * TG + 128 * s
                at = ats[s]
                ps = scratch_ps(F32)
                for c in range(PAIRS):
                    nc.tensor.matmul(
                        ps[:, :512], at[:, 128 * c:128 * (c + 1)],
                        wout_sb[c][:, 512 * nh:512 * (nh + 1)],
                        start=(c == 0), stop=(c == PAIRS - 1))
                ob = ob_pool.tile([128, 512], F32, tag="ob")
                if _ps_state["wide"] and (s + nh) % 2 == 0:
                    nc.scalar.copy(ob[:], ps[:, :512])
                else:
                    nc.vector.tensor_copy(out=ob[:], in_=ps[:, :512])
                nc.sync.dma_start(
                    out=out[row0:row0 + 128, 512 * nh:512 * (nh + 1)],
                    in_=ob[:])

            ounits = []
            for s in range(TG // 128):
                ounits.append(lambda s=s: trans_unit(s))
            for s in range(TG // 128):
                for nh in range(2):
                    ounits.append(lambda s=s, nh=nh: oproj_unit(s, nh))
            return units, ounits

        def interleave(a_units, b_units):
            na, nb = len(a_units), len(b_units)
            ia = ib = 0
            while ia < na or ib < nb:
                fa = (na - ia) / na if na else 0.0
                fb = (nb - ib) / nb if nb else 0.0
                if ia < na and (fa >= fb or ib >= nb):
                    a_units[ia]()
                    ia += 1
                else:
                    b_units[ib]()
                    ib += 1

        # prologue: group 0 projection (weight DMAs after group 0's x loads).
        # The scores/AV banks are idle here, so scratch rotates through them.
        set_wide_scratch(True)
        # fill the initial x/w DMA latency with dummy PE work (also completes
        # the tensor engine's p-state ramp before real work lands); plain
        # matmuls on a memset tile need no identity, so they start ~1us in
        dummy = const.tile([128, 512], BF16, tag="dummy")
        nc.vector.memset(dummy[:], 0.0)
        for _ in range(6):
            wps = scratch_ps(F32)
            nc.tensor.matmul(wps[:, :512], dummy[:, 0:128], dummy[:],
                             start=True, stop=True)
        xt0, tunits0 = transpose_units(0)
        for u in tunits0:
            u()
        load_weights()
        q0, kv0 = qkv_units(0, xt0)
        for u in q0 + kv0:
            u()
        set_wide_scratch(False)
        # steady state: attention(g) interleaved with transposes(g+1) +
        # projection(g+1); out-projections are deferred up to two groups so
        # the last (largest) attention group still has dense PE fill
        pending_oproj = []  # deferred out-projection unit lists, oldest first
        kv_pending = []     # group g's own kT/V chains, deferred to phase g
        for g in range(n_tg):
            attn, ounits = attention_units(g)
            fill = []
            if g + 1 < n_tg:
                xt1, tunits = transpose_units(g + 1)
                q1, kv1 = qkv_units(g + 1, xt1)
                fill += tunits + q1 + kv1
            else:
                # last group is exp(ACT)-bound and has no next-group
                # projection: feed it ALL deferred out-projections as fill
                while pending_oproj:
                    fill += pending_oproj.pop(0)
            interleave(attn, fill)
            pending_oproj.append(ounits)
        # final drain: attention is done, scores/AV banks are idle again
        set_wide_scratch(True)
        for ou in pending_oproj:
            for u in ou:
                u()


def build_program(with_bias, seq_len=T):
    nc = bacc.Bacc("TRN2", target_bir_lowering=False, debug=False,
                   enable_asserts=False, num_devices=8)
    x = nc.dram_tensor("xt", [D, seq_len], BF16, kind="ExternalInput").ap()
    wqkv = nc.dram_tensor("wqkv", [D, 3 * HPC * HD], BF16,
                          kind="ExternalInput").ap()
    wout = nc.dram_tensor("wout", [HPC * HD, D], BF16, kind="ExternalInput").ap()
    out = nc.dram_tensor("out", [seq_len, D], F32, kind="ExternalOutput").ap()
    bqkv = None
    if with_bias:
        bqkv = nc.dram_tensor("bqkv", [1, 3 * HPC * HD], F32,
                              kind="ExternalInput").ap()
    with tile.TileContext(nc) as tc:
        build_tile_program(tc, x, wqkv, wout, out, bqkv, seq_len=seq_len)
    nc.compile()
    return nc


_PROGRAM_CACHE = {}


def _get_program(with_bias):
    if with_bias not in _PROGRAM_CACHE:
        _PROGRAM_CACHE[with_bias] = build_program(with_bias)
    return _PROGRAM_CACHE[with_bias]


def make_in_maps(x, w_qkv, b_qkv, w_out, with_bias):
    """Per-core input dicts: core c -> batch c//2, head group c%2."""
    in_maps = []
    for core in range(8):
        b, gr = divmod(core, 2)
        qc = slice(512 * gr, 512 * (gr + 1))
        kc = slice(D + 512 * gr, D + 512 * (gr + 1))
        vc = slice(2 * D + 512 * gr, 2 * D + 512 * (gr + 1))
        wq = np.ascontiguousarray(
            np.concatenate([w_qkv[:, qc], w_qkv[:, kc], w_qkv[:, vc]], axis=1))
        import ml_dtypes
        m = {
            "xt": np.ascontiguousarray(x[b].T).astype(ml_dtypes.bfloat16),
            "wqkv": wq.astype(ml_dtypes.bfloat16),
            "wout": np.ascontiguousarray(
                w_out[512 * gr:512 * (gr + 1), :]).astype(ml_dtypes.bfloat16),
        }
        if with_bias:
            m["bqkv"] = np.ascontiguousarray(
                np.concatenate([b_qkv[qc], b_qkv[kc], b_qkv[vc]])
            ).reshape(1, -1)
        in_maps.append(m)
    return in_maps


def kernel(x, w_qkv, b_qkv, w_out, b_out):
    x = np.asarray(x, dtype=np.float32)
    w_qkv = np.asarray(w_qkv, dtype=np.float32)
    b_qkv = np.asarray(b_qkv, dtype=np.float32)
    w_out = np.asarray(w_out, dtype=np.float32)
    b_out = np.asarray(b_out, dtype=np.float32)

    with_bias = bool(np.any(b_qkv))
    nc = _get_program(with_bias)
    in_maps = make_in_maps(x, w_qkv, b_qkv, w_out, with_bias)
    res = run_bass_kernel_spmd(nc, in_maps, core_ids=list(range(8))).results

    out = np.empty((B, T, D), dtype=np.float32)
    for b in range(B):
        out[b] = res[2 * b]["out"] + res[2 * b + 1]["out"] + b_out[None, :]
    return out


# revision 85
# speedup vs baseline: 1.3711x; 1.0316x over previous
"""Causal self-attention (B=4, T=2048, D=1024, H=16, hd=64) on 8 trn2 cores.

Sharding: 4-way data parallel over batch x 2-way tensor parallel over heads.
Core c handles batch c//2 and heads [8*(c%2), 8*(c%2)+8). Each core computes
its heads' partial contribution to the out-projection (a full [T, D] tensor);
the host sums the two head-group partials per batch and adds b_out.

Host-side prep (layout only): x is transposed to xT [D, T] and converted to
bf16, w_qkv is regrouped per core and converted to bf16, w_out likewise.
End-to-end rel err vs the fp32 reference is ~5e-3 (gate: 2e-2).

Per-core dataflow (single NeuronCore, Tile framework):
  1. xT streams straight from DRAM into SBUF (no on-device transposes).
  2. qT/kT [hd,T] are computed head-PAIR-stacked ([128,T] per pair) with
     w_qkv chunks as stationary; V [T,hd] per head is computed naturally and
     scattered into v1 (64 cols per (head, k-tile), bf16).
  3. Scores are computed TRANSPOSED: S^T[k,q] = K @ Q^T via two K=64 matmuls
     per (pair, k-tile) into one [128, 2, 512] two-bank PSUM tile; ONE wide
     exp (ACT, scale folded in) produces P^T for both heads in bf16.
     Causality: k-tiles above the diagonal are skipped, diagonal blocks get
     an affine_select zero-mask after exp, partial k-tiles use narrower
     matmuls.
  4. AV runs NON-transposed: per (head, q-subtile 128) the P^T block is the
     STATIONARY operand and V [128,64] streams, so each matmul moves 64
     rows instead of 512. All 8 (head, subtile) accumulators of a pair pack
     into ONE psum bank (zero-on-first-touch after a single start=True).
     Softmax sums ride as 1-column matmuls (P^T stationary, ones moving)
     into a shared, DVE-zeroed sums bank; AV emission trails its exp by 8
     k-tile units (carry) so the PE never waits on ACT latency.
  5. Normalization is a DVE broadcast multiply (reciprocal of sums, then
     attn * recip -> bf16). The normalized attn [q, 512] is PE-transposed
     (bf16) into the out-projection's stationary operand; out = attnT.T @
     w_out accumulates over 4 hd-chunks. All out-projections are deferred
     into the last q-group's phase, which is otherwise exp(ACT)-bound.

Scheduling: emission-level software pipelining interleaves attention for
q-group g with the projection of group g+1. PSUM (8 banks) is budgeted as
4 scores + 2 AV + 1 sums + 1 projection scratch; in the prologue and final
drain the idle scores/AV banks widen the projection scratch rotation to 5.
A few dummy matmuls at t=0 cover the first DMA latency and finish the PE
p-state ramp before real work lands.
"""

import numpy as np

import concourse.bass as bass
import concourse.mybir as mybir
import concourse.tile as tile
from concourse import bacc
from concourse.bass_utils import run_bass_kernel_spmd
from concourse.masks import make_identity

B, T, D = 4, 2048, 1024
H, HD = 16, 64
HPC = 8              # heads per core
PAIRS = HPC // 2
CH = D // 128        # K-chunks for the QKV projection
TG = 512             # T-group / q-group width
SCALE = 1.0 / 8.0    # 1/sqrt(HD)

F32 = mybir.dt.float32
F32R = mybir.dt.float32r
BF16 = mybir.dt.bfloat16
AF = mybir.ActivationFunctionType


def _r(ap):
    return ap.bitcast(F32R)


def build_tile_program(tc, x, wqkv, wout, out, bqkv=None, seq_len=T):
    """Emit the per-core program. seq_len is parametrized for small-scale
    simulation tests; the real kernel uses seq_len=T=2048.

    Emission is software-pipelined: attention for q-group g (latency-bound
    serial chains sT -> exp -> mask -> AV) is interleaved at emission level
    with the transposes + QKV projection of group g+1 (dense PE work), so the
    in-order PE fills attention's dependency stalls with projection matmuls.
    """
    nc = tc.nc
    n_tg = seq_len // TG
    n_tt = seq_len // 128
    with_bias = bqkv is not None

    from contextlib import ExitStack
    with ExitStack() as ctx:
        const = ctx.enter_context(tc.tile_pool(name="const", bufs=1))
        wpool = ctx.enter_context(tc.tile_pool(name="wpool", bufs=1))
        xt_pool = ctx.enter_context(tc.tile_pool(name="xt", bufs=2))
        qt_pool = ctx.enter_context(tc.tile_pool(name="qt", bufs=2 * PAIRS))
        pt_pool = ctx.enter_context(tc.tile_pool(name="pt", bufs=14))
        an_pool = ctx.enter_context(tc.tile_pool(name="an", bufs=16))
        at_pool = ctx.enter_context(tc.tile_pool(name="at", bufs=8))
        ob_pool = ctx.enter_context(tc.tile_pool(name="ob", bufs=6))
        rc_pool = ctx.enter_context(tc.tile_pool(name="rc", bufs=2))
        mm_ps = ctx.enter_context(tc.tile_pool(name="mmps", bufs=1, space="PSUM"))
        st_ps = ctx.enter_context(tc.tile_pool(name="stps", bufs=2, space="PSUM"))
        av_ps = ctx.enter_context(tc.tile_pool(name="avps", bufs=2, space="PSUM"))
        sums_ps = ctx.enter_context(tc.tile_pool(name="sums", bufs=1, space="PSUM"))
        identity = const.tile([128, 128], F32, tag="ident")
        ident_bf = const.tile([128, 128], BF16, tag="identbf")
        ones_bf = const.tile([128, 1], BF16, tag="ones_bf")

        def make_consts():
            # deferred so the weight SWDGE generation isn't queued behind
            # make_identity on the Pool engine at t=0; the identity is only
            # needed by the (late) attnT transposes
            make_identity(nc, identity[:])
            nc.vector.tensor_copy(out=ident_bf[:], in_=identity[:])
            nc.vector.memset(ones_bf[:], 1.0)
        if with_bias:
            ones_f = const.tile([128, 64], F32, tag="ones_f")
            nc.vector.memset(ones_f[:], 1.0)
            b_sb = const.tile([1, 3 * HPC * HD], F32R, tag="bias")
            nc.sync.dma_start(out=b_sb[:], in_=bqkv.bitcast(F32R)[:])
            ones_row = const.tile([1, TG], F32R, tag="ones_row")
            nc.vector.tensor_copy(
                out=ones_row[:], in_=ones_f[0:1, 0:1].broadcast_to([1, TG]))

        # softmax sums: one persistent bank; cols = parity*32 + pair*8 + h*4+s
        sums = sums_ps.tile([128, 64], F32, tag="sums")

        # resident weights; w_out pair-packed: pair p rows [128p, 128p+128).
        # The DMAs are deferred until after group 0's x loads so the first
        # transposes aren't queued behind 8MB of weight traffic.
        w_sb = wpool.tile([128, CH, 3 * HPC * HD], BF16, tag="wqkv")
        wout_sb = [wpool.tile([128, D], BF16, tag=f"woutp{p}", name=f"woutp{p}")
                   for p in range(PAIRS)]

        def load_weights():
            # SWDGE queue (gpsimd): runs in parallel with the sync-engine x
            # loads; q|k column blocks first (consumed first by qt/kT chains)
            for c in (0, 1):
                nc.gpsimd.dma_start(
                    out=w_sb[:, c, 0:1024],
                    in_=wqkv[128 * c:128 * (c + 1), 0:1024])
            for c0 in range(2, CH, 2):
                nc.gpsimd.dma_start(
                    out=w_sb[:, c0:c0 + 2, 0:1024],
                    in_=wqkv[128 * c0:128 * (c0 + 2), 0:1024].rearrange(
                        "(c p) n -> p c n", p=128))
            for c0 in range(0, CH, 4):
                nc.gpsimd.dma_start(
                    out=w_sb[:, c0:c0 + 4, 1024:1536],
                    in_=wqkv[128 * c0:128 * (c0 + 4), 1024:1536].rearrange(
                        "(c p) n -> p c n", p=128))
            for p in range(PAIRS):
                nc.gpsimd.dma_start(
                    out=wout_sb[p][:],
                    in_=wout[128 * p:128 * (p + 1), :])
        # kT per pair, head 2p on partitions [0:64), head 2p+1 on [64:128)
        kts = [wpool.tile([128, seq_len], BF16, tag=f"kt{p}", name=f"kt{p}")
               for p in range(PAIRS)]
        # V in bf16: per (head, k-tile) a [128, 64] stationary block
        v1 = wpool.tile([128, HPC, n_tt, HD], BF16, tag="v1")

        qts_of = {}  # g -> [qt tiles per pair]

        # psum scratch for the projection/transpose/oproj chains. In steady
        # state only the mm bank is free; in the prologue and final drain the
        # scores/AV banks are idle, so rotate through them too (the st slot is
        # [128,2,512]; its first bank is used as a [128,512] scratch).
        _ps_state = {"wide": False, "i": 0}

        def set_wide_scratch(wide):
            _ps_state["wide"] = wide

        def scratch_ps(dtype):
            if not _ps_state["wide"]:
                return mm_ps.tile([128, 512], dtype, tag="mm", name="mm")
            i = _ps_state["i"] = (_ps_state["i"] + 1) % 5
            if i == 0:
                return mm_ps.tile([128, 512], dtype, tag="mm", name="mm")
            if i in (1, 2):
                return st_ps.tile([128, 512], dtype, tag="st", name="stx")
            return av_ps.tile([128, 512], dtype, tag="av", name="avx")

        def transpose_units(g):
            """x arrives pre-transposed from the host: just DMA the group's
            xT columns, one [128, TG] block per D-chunk."""
            xt = xt_pool.tile([128, CH, TG], BF16, tag="xt", name=f"xt{g}")

            def u():
                for c in range(CH):
                    nc.sync.dma_start(
                        out=xt[:, c, :],
                        in_=x[128 * c:128 * (c + 1), g * TG:(g + 1) * TG])
            return xt, [u]

        def qkv_units(g, xt):
            """12 units: 4 qt chains, 4 kT chains, 4 V chains."""
            qts = qts_of.setdefault(g, [])
            units = []

            half = {}

            def qk_chain(p, qk, h):
                # split into two half-chains (finer interleave granularity)
                if h == 0:
                    half[(p, qk)] = scratch_ps(F32)
                ps = half[(p, qk)]
                col = qk * 512 + p * 128
                for c in range(4 * h, 4 * h + 4):
                    nc.tensor.matmul(
                        ps[:, :TG], w_sb[:, c, col:col + 128], xt[:, c, :],
                        start=(c == 0),
                        stop=(c == CH - 1 and not with_bias))
                if h == 0:
                    return
                if with_bias:
                    nc.tensor.matmul(
                        ps[:, :TG], b_sb[0:1, col:col + 128],
                        ones_row[0:1, :], start=False, stop=True)
                if qk == 0:
                    qt = qt_pool.tile([128, TG], BF16, tag="qt")
                    nc.vector.tensor_copy(out=qt[:], in_=ps[:, :TG])
                    qts.append(qt)
                else:
                    nc.vector.tensor_copy(
                        out=kts[p][:, g * TG:(g + 1) * TG], in_=ps[:, :TG])

            def v_chain(t4):
                tt = g * (TG // 128) + t4
                ps = scratch_ps(F32)
                for c in range(CH):
                    nc.tensor.matmul(
                        ps[:, :512], xt[:, c, 128 * t4:128 * (t4 + 1)],
                        w_sb[:, c, 1024:1536],
                        start=(c == 0),
                        stop=(c == CH - 1 and not with_bias))
                if with_bias:
                    nc.tensor.matmul(
                        ps[:, :512], ones_row[0:1, 0:128],
                        b_sb[0:1, 1024:1536], start=False, stop=True)
                nc.vector.tensor_copy(
                    out=v1[:, :, tt, :],
                    in_=ps[:, :512].rearrange("p (h d) -> p h d", h=HPC))

            kv_units = []
            # in the prologue the qk chains are gated on their W chunks
            # landing (~1us apart): run all h=0 half-chains (chunks 0-3)
            # before any h=1 (chunks 4-7). Requires 4 concurrent psum
            # accumulators - only legal in the wide-scratch prologue.
            hmajor = _ps_state["wide"]
            for h in range(2):
                for p in range(PAIRS):
                    units.append(lambda p=p, h=h: qk_chain(p, 0, h))
                if not hmajor and h == 0:
                    units = units[:-PAIRS]
                    for p in range(PAIRS):
                        units.append(lambda p=p: qk_chain(p, 0, 0))
                        units.append(lambda p=p: qk_chain(p, 0, 1))
                    break
            for h in range(2):
                for p in range(PAIRS):
                    kv_units.append(lambda p=p, h=h: qk_chain(p, 1, h))
                if not hmajor and h == 0:
                    kv_units = kv_units[:-PAIRS]
                    for p in range(PAIRS):
                        kv_units.append(lambda p=p: qk_chain(p, 1, 0))
                        kv_units.append(lambda p=p: qk_chain(p, 1, 1))
                    break
            for t4 in range(TG // 128):
                kv_units.append(lambda t4=t4: v_chain(t4))
            return units, kv_units

        def attention_units(g):
            """Per pair: one unit per k-tile (sT+exp+mask, AV carried by one),
            then a normalization unit; finally the out-projection units."""
            units = []
            pending_finish = []
            an_tiles = [an_pool.tile([128, 512], BF16, tag="an",
                                     name=f"an_g{g}_s{s}")
                        for s in range(TG // 128)]
            # zero this group's sums columns once (all 4 pairs' 8-col slices)
            nc.vector.memset(sums[:, (g % 2) * 32:(g % 2) * 32 + 32], 0.0)
            qts = qts_of[g]
            for p in range(PAIRS):
                nkt = 4 * (g + 1)
                soff = (g % 2) * 32 + p * 8
                state = {}

                def start_pair(p=p, state=state, soff=soff):
                    state["av"] = av_ps.tile([128, 512], F32, tag="av",
                                             name=f"av_g{g}_p{p}")
                    state["carry"] = []
                    state["first"] = True

                def kt_unit(kt, pos, p=p, state=state, nkt=nkt,
                            sp=start_pair):
                    if pos == 0:
                        sp()
                    # pop the carried AV FIRST: it is ready now, and the
                    # scores below may head-block on a busy st slot
                    if len(state["carry"]) >= 8:
                        state["emit_av"](*state["carry"].pop(0))
                    qt = qts[p]
                    rdiag = kt - 4 * g
                    col0 = 128 * rdiag if rdiag > 0 else 0
                    ksl = slice(128 * kt, 128 * (kt + 1))
                    st = st_ps.tile([128, 2, 512], F32, tag="st")
                    nc.tensor.matmul(
                        st[:, 0, col0:], kts[p][0:64, ksl], qt[0:64, col0:])
                    nc.tensor.matmul(
                        st[:, 1, col0:], kts[p][64:128, ksl], qt[64:128, col0:])
                    pt = pt_pool.tile([128, 2, 512], BF16, tag="pt")
                    nc.scalar.activation(
                        pt[:, :, col0:], st[:, :, col0:], AF.Exp, scale=SCALE)
                    if rdiag >= 0:
                        for h in (0, 1):
                            # keep P^T[k, q] only where q >= k (within-block)
                            nc.gpsimd.affine_select(
                                out=pt[:, h, col0:col0 + 128],
                                in_=pt[:, h, col0:col0 + 128],
                                compare_op=mybir.AluOpType.is_ge,
                                fill=0.0, base=0, pattern=[[1, 128]],
                                channel_multiplier=-1)
                    state["carry"].append((kt, pos, rdiag, pt))

                def emit_av(kt, pos, rdiag, pt, p=p, state=state, nkt=nkt,
                            soff=soff):
                    av = state["av"]
                    s0 = max(rdiag, 0)
                    last = pos == nkt - 1
                    for s in range(s0, TG // 128):
                        for h in (0, 1):
                            nc.tensor.matmul(
                                av[:, 128 * s + 64 * h:128 * s + 64 * h + 64],
                                pt[:, h, 128 * s:128 * (s + 1)],
                                v1[:, 2 * p + h, kt, :],
                                start=state["first"],
                                stop=(last and s == 3 and h == 1))
                            state["first"] = False
                            nc.tensor.matmul(
                                sums[:, soff + 4 * h + s:soff + 4 * h + s + 1],
                                pt[:, h, 128 * s:128 * (s + 1)],
                                ones_bf[:],
                                start=False, stop=False,
                                skip_group_check=True)

                state["emit_av"] = emit_av

                def emit_block(mm, state, p, soff, stop_last):
                    av = state["av"]
                    for i, (isd, kt, pt, s, h) in enumerate(mm):
                        nc.tensor.matmul(
                            av[:, 128 * s + 64 * h:128 * s + 64 * h + 64],
                            pt[:, h, 128 * s:128 * (s + 1)],
                            v1[:, 2 * p + h, kt, :],
                            start=state["first"],
                            stop=(stop_last and i == len(mm) - 1))
                        state["first"] = False
                        nc.tensor.matmul(
                            sums[:, soff + 4 * h + s:soff + 4 * h + s + 1],
                            pt[:, h, 128 * s:128 * (s + 1)],
                            ones_bf[:],
                            start=False, stop=False,
                            skip_group_check=True)

                def flush_unit(p=p, state=state, soff=soff):
                    # flush the carried non-diagonal AVs at pair end; the
                    # affine-masked diagonal blocks wait for Pool latency, so
                    # they are deferred into finish_unit (emitted two k-tile
                    # units into the NEXT pair) to avoid head-blocking the
                    # PE's 4-deep dependency wait queue
                    mm = []
                    for kt, pos, rdiag, pt in state["carry"]:
                        for s in range(max(rdiag, 0), TG // 128):
                            for h in (0, 1):
                                mm.append((s == rdiag, kt, pt, s, h))
                    state["carry"] = []
                    mm.sort(key=lambda t: t[0])
                    ndiag = sum(1 for t in mm if t[0])
                    split = len(mm) - ndiag
                    emit_block(mm[:split], state, p, soff, stop_last=False)
                    state["diag"] = mm[split:]

                def norm_unit(p=p, state=state, soff=soff):
                    emit_block(state.pop("diag"), state, p, soff,
                               stop_last=True)
                    av = state["av"]
                    rc = rc_pool.tile([128, 2, 4], F32, tag="rc")
                    nc.vector.reciprocal(
                        rc[:], sums[:, soff:soff + 8].rearrange(
                            "p (h s) -> p h s", h=2))
                    for s in range(TG // 128):
                        nc.vector.tensor_mul(
                            an_tiles[s][:, 128 * p:128 * (p + 1)].rearrange(
                                "p (h d) -> p h d", h=2),
                            av[:, 128 * s:128 * (s + 1)].rearrange(
                                "p (h d) -> p h d", h=2),
                            rc[:, :, s:s + 1].broadcast_to([128, 2, 64]))

                pair_units = []
                for pos, kt in enumerate(range(nkt)):
                    pair_units.append(
                        lambda kt=kt, pos=pos, f=kt_unit: f(kt, pos))
                if pending_finish:
                    pair_units.insert(min(2, len(pair_units)),
                                      pending_finish.pop())
                units += pair_units
                units.append(flush_unit)
                pending_finish.append(norm_unit)

            if pending_finish:
                units.append(pending_finish.pop())

            ats = {}

            def trans_unit(s):
                ps = scratch_ps(BF16)
                for c in range(PAIRS):
                    nc.tensor.transpose(
                        ps[:, 128 * c:128 * (c + 1)],
                        an_tiles[s][:, 128 * c:128 * (c + 1)],
                        ident_bf[:])
                at = at_pool.tile([128, 512], BF16, tag="at")
                nc.vector.tensor_copy(out=at[:], in_=ps[:])
                ats[s] = at

            def oproj_unit(s, nh):
                row0 = g * TG + 128 * s
                at = ats[s]
                ps = scratch_ps(F32)
                for c in range(PAIRS):
                    nc.tensor.matmul(
                        ps[:, :512], at[:, 128 * c:128 * (c + 1)],
                        wout_sb[c][:, 512 * nh:512 * (nh + 1)],
                        start=(c == 0), stop=(c == PAIRS - 1))
                ob = ob_pool.tile([128, 512], BF16, tag="ob")
                if _ps_state["wide"] and (s + nh) % 2 == 0:
                    nc.scalar.copy(ob[:], ps[:, :512])
                else:
                    nc.vector.tensor_copy(out=ob[:], in_=ps[:, :512])
                nc.sync.dma_start(
                    out=out[row0:row0 + 128, 512 * nh:512 * (nh + 1)],
                    in_=ob[:])

            ounits = []
            for s in range(TG // 128):
                ounits.append(lambda s=s: trans_unit(s))
            for s in range(TG // 128):
                for nh in range(2):
                    ounits.append(lambda s=s, nh=nh: oproj_unit(s, nh))
            return units, ounits

        def interleave(a_units, b_units):
            na, nb = len(a_units), len(b_units)
            ia = ib = 0
            while ia < na or ib < nb:
                fa = (na - ia) / na if na else 0.0
                fb = (nb - ib) / nb if nb else 0.0
                if ia < na and (fa >= fb or ib >= nb):
                    a_units[ia]()
                    ia += 1
                else:
                    b_units[ib]()
                    ib += 1

        # prologue: group 0 projection (weight DMAs after group 0's x loads).
        # The scores/AV banks are idle here, so scratch rotates through them.
        set_wide_scratch(True)
        # fill the initial x/w DMA latency with dummy PE work (also completes
        # the tensor engine's p-state ramp before real work lands); plain
        # matmuls on a memset tile need no identity, so they start ~1us in
        dummy = const.tile([128, 256], BF16, tag="dummy")
        nc.vector.memset(dummy[:], 0.0)
        for _ in range(12):
            wps = scratch_ps(F32)
            nc.tensor.matmul(wps[:, :256], dummy[:, 0:128], dummy[:],
                             start=True, stop=True)
        xt0, tunits0 = transpose_units(0)
        for u in tunits0:
            u()
        load_weights()
        q0, kv0 = qkv_units(0, xt0)
        for u in q0 + kv0:
            u()
        set_wide_scratch(False)
        # steady state: attention(g) interleaved with transposes(g+1) +
        # projection(g+1); out-projections are deferred up to two groups so
        # the last (largest) attention group still has dense PE fill
        pending_oproj = []  # deferred out-projection unit lists, oldest first
        kv_pending = []     # group g's own kT/V chains, deferred to phase g
        for g in range(n_tg):
            attn, ounits = attention_units(g)
            fill = []
            if g + 1 < n_tg:
                xt1, tunits = transpose_units(g + 1)
                q1, kv1 = qkv_units(g + 1, xt1)
                fill += tunits + q1 + kv1
            else:
                # last group is exp(ACT)-bound and has no next-group
                # projection: feed it ALL deferred out-projections as fill
                while pending_oproj:
                    fill += pending_oproj.pop(0)
            interleave(attn, fill)
            pending_oproj.append(ounits)
        # final drain: attention is done, scores/AV banks are idle again
        set_wide_scratch(True)
        for ou in pending_oproj:
            for u in ou:
                u()


def build_program(with_bias, seq_len=T):
    nc = bacc.Bacc("TRN2", target_bir_lowering=False, debug=False,
                   enable_asserts=False, num_devices=8)
    x = nc.dram_tensor("xt", [D, seq_len], BF16, kind="ExternalInput").ap()
    wqkv = nc.dram_tensor("wqkv", [D, 3 * HPC * HD], BF16,
                          kind="ExternalInput").ap()
    wout = nc.dram_tensor("wout", [HPC * HD, D], BF16, kind="ExternalInput").ap()
    out = nc.dram_tensor("out", [seq_len, D], BF16,
                         kind="ExternalOutput").ap()
    bqkv = None
    if with_bias:
        bqkv = nc.dram_tensor("bqkv", [1, 3 * HPC * HD], F32,
                              kind="ExternalInput").ap()
    with tile.TileContext(nc) as tc:
        build_tile_program(tc, x, wqkv, wout, out, bqkv, seq_len=seq_len)
    nc.compile()
    return nc


_PROGRAM_CACHE = {}


def _get_program(with_bias):
    if with_bias not in _PROGRAM_CACHE:
        _PROGRAM_CACHE[with_bias] = build_program(with_bias)
    return _PROGRAM_CACHE[with_bias]


def make_in_maps(x, w_qkv, b_qkv, w_out, with_bias):
    """Per-core input dicts: core c -> batch c//2, head group c%2."""
    in_maps = []
    for core in range(8):
        b, gr = divmod(core, 2)
        qc = slice(512 * gr, 512 * (gr + 1))
        kc = slice(D + 512 * gr, D + 512 * (gr + 1))
        vc = slice(2 * D + 512 * gr, 2 * D + 512 * (gr + 1))
        wq = np.ascontiguousarray(
            np.concatenate([w_qkv[:, qc], w_qkv[:, kc], w_qkv[:, vc]], axis=1))
        import ml_dtypes
        m = {
            "xt": np.ascontiguousarray(x[b].T).astype(ml_dtypes.bfloat16),
            "wqkv": wq.astype(ml_dtypes.bfloat16),
            "wout": np.ascontiguousarray(
                w_out[512 * gr:512 * (gr + 1), :]).astype(ml_dtypes.bfloat16),
        }
        if with_bias:
            m["bqkv"] = np.ascontiguousarray(
                np.concatenate([b_qkv[qc], b_qkv[kc], b_qkv[vc]])
            ).reshape(1, -1)
        in_maps.append(m)
    return in_maps


def kernel(x, w_qkv, b_qkv, w_out, b_out):
    x = np.asarray(x, dtype=np.float32)
    w_qkv = np.asarray(w_qkv, dtype=np.float32)
    b_qkv = np.asarray(b_qkv, dtype=np.float32)
    w_out = np.asarray(w_out, dtype=np.float32)
    b_out = np.asarray(b_out, dtype=np.float32)

    with_bias = bool(np.any(b_qkv))
    nc = _get_program(with_bias)
    in_maps = make_in_maps(x, w_qkv, b_qkv, w_out, with_bias)
    res = run_bass_kernel_spmd(nc, in_maps, core_ids=list(range(8))).results

    out = np.empty((B, T, D), dtype=np.float32)
    for b in range(B):
        out[b] = (res[2 * b]["out"].astype(np.float32)
                  + res[2 * b + 1]["out"].astype(np.float32)
                  + b_out[None, :])
    return out
